# revision 26
# baseline (speedup 1.0000x reference)
"""Trainium2 Bass kernel for nn_Net_48301202211072 (GNN message passing).

2-layer GraphConv + TopKPooling + readout + MLP head, sharded over 8
NeuronCores. v2 design:

- Nodes sharded by dst: core c owns nodes [c*1250, (c+1)*1250), split into
  10 contiguous bins of 125 dsts (each bin maps to 128 PSUM partitions).
- Aggregation: per bin, dma_gather raw bf16 source rows (one slot per
  edge, trailing -1 pads skipped via per-core runtime counts), build a
  weighted one-hot [slots, dst] on DVE (2 ops/bin), contract on the PE:
  agg[dst, F] += oh.T @ gathered, accumulated in PSUM across chunks.
- Dense, scores, topk threshold (replicated 5-stage 64-bin histogram,
  contiguous-reduce layout), masked readout: same structure as v1 but
  bf16 operands for all matmuls and gathers.
- g1 table (layer-2 gather source) and its AllGather are bf16.
- Head: lin1 row-sharded; lin2 partial-contraction + one [1,4096]
  AllReduce; lin3 replicated (no final collective).
"""
import dataclasses
import math
import sys

import ml_dtypes
import numpy as np

sys.path.insert(0, "/opt/trn_rl_repo")

import concourse.bacc as bacc  # noqa: E402
import concourse.mybir as mybir  # noqa: E402
import concourse.tile as tile  # noqa: E402
from concourse import bass_utils  # noqa: E402

FP32 = mybir.dt.float32
BF16 = mybir.dt.bfloat16
I16 = mybir.dt.int16
I32 = mybir.dt.int32
AX = mybir.AxisListType
OP = mybir.AluOpType
ACT = mybir.ActivationFunctionType
BF = ml_dtypes.bfloat16

NCORES = 8
N = 10000
FIN = 256
HID = 500
HPAD = 512
NOUT = 100
NPC = N // NCORES          # 1250 nodes per core
NCH = 10                   # bins per core
BINW = NPC // NCH          # 125 dsts per bin
NPAD = NCH * 128           # 1280 padded rows per core
NBINS = 64
NSTAGES = 3
K1 = N // 2
K2 = N // 4
BIG = 1e30


def _bininfo(node):
    """global node id -> (source group 0/1, row in the full AG table)."""
    c = node // NPC
    d = node % NPC
    b = d // BINW
    g = (b >= NCH // 2).astype(np.int64)
    return g, c * NPAD + b * 128 + d % BINW


# ---------------------------------------------------------------------------
# host preprocessing
# ---------------------------------------------------------------------------

def _pack(edge_src, edge_dst, edge_weight):
    src = np.asarray(edge_src, np.int64)
    dst = np.asarray(edge_dst, np.int64)
    w = np.asarray(edge_weight, np.float32)

    bins = []  # [core][bin][grp] -> (src_ids, dst_in_bin, w)
    for c in range(NCORES):
        lo = c * NPC
        m = (dst >= lo) & (dst < lo + NPC)
        es, ed, ew = src[m], dst[m] - lo, w[m]
        cb = []
        for b in range(NCH):
            mb = (ed >= b * BINW) & (ed < (b + 1) * BINW)
            bes, bed, bew = es[mb], ed[mb] - b * BINW, ew[mb]
            sg = (bes % NPC) // BINW >= NCH // 2   # source group A/B
            cb.append([(bes[~sg], bed[~sg], bew[~sg]),
                       (bes[sg], bed[sg], bew[sg])])
        bins.append(cb)

    def chmax(b, g):
        return max(1, max((len(bins[c][b][g][0]) + 127) // 128
                          for c in range(NCORES)))
    ncha = tuple(chmax(b, 0) for b in range(NCH))
    nchbb = tuple(chmax(b, 1) for b in range(NCH))
    return dict(ncha=ncha, nchbb=nchbb,
                nchb=tuple(a + bb for a, bb in zip(ncha, nchbb))), bins


def _wrap16(flat):
    """[L] int array (L % 128 == 0) -> [128, L//16] int16 wrapped-16."""
    L = len(flat)
    t = np.asarray(flat, np.int16).reshape(L // 16, 16).T  # [16, L//16]
    return np.ascontiguousarray(np.tile(t, (8, 1)))


def _host_inputs(inputs, cfg, bins):
    nchb = cfg["nchb"]
    totch = sum(nchb)
    x = np.asarray(inputs["x"], np.float32)
    xbf = x.astype(BF)

    def padT(a, rows, cols, dt=BF):
        out = np.zeros((rows, cols), dt)
        t = np.asarray(a, np.float32).T
        out[: t.shape[0], : t.shape[1]] = t.astype(dt)
        return out

    w1relT = padT(inputs["W1_rel"], FIN, HPAD)
    w1rootT = padT(inputs["W1_root"], FIN, HPAD)
    w2relT = padT(inputs["W2_rel"], HPAD, HPAD)
    w2rootT = padT(inputs["W2_root"], HPAD, HPAD)

    def repl(v, cols):
        out = np.zeros((128, cols), np.float32)
        vv = np.asarray(v, np.float32)
        out[:, : vv.shape[0]] = vv[None, :]
        return out

    b1r = repl(inputs["b1"], HPAD)
    b2r = repl(inputs["b2"], HPAD)
    p1r = repl(inputs["p1_w"], HPAD).astype(BF)
    p2r = repl(inputs["p2_w"], HPAD).astype(BF)

    iota128 = np.tile(np.arange(128, dtype=np.float32)[None, :], (128, 1))
    iotaB = np.tile(np.arange(NBINS, dtype=np.float32)[None, :], (128, 1))
    ident = np.eye(128, dtype=np.float32)
    identbf = np.eye(128, dtype=BF)
    ones1x128 = np.ones((1, 128), np.float32)
    ones128 = np.ones((128, 128), np.float32)
    onesP = np.ones((128, 1), np.float32)
    onesPbf = np.ones((128, 1), BF)

    # padmask [128, NCH]: partition p of any bin is real iff p < BINW
    padmask = np.zeros((128, NCH), np.float32)
    padmask[:BINW, :] = 1.0

    lin1W = np.asarray(inputs["lin1_W"], np.float32)   # [2000, 1000]
    lin2W = np.asarray(inputs["lin2_W"], np.float32)   # [4000, 2000]
    lin3W = np.asarray(inputs["lin3_W"], np.float32)   # [100, 4000]
    lin1b = np.asarray(inputs["lin1_b"], np.float32)
    lin2b = np.asarray(inputs["lin2_b"], np.float32)
    lin3b = np.asarray(inputs["lin3_b"], np.float32)

    # lin3: replicated [4096, 100] bf16
    l3T = np.zeros((4096, 128), BF)
    l3T[:4000, :NOUT] = lin3W.T.astype(BF)
    b3row = np.zeros((1, 128), np.float32)
    b3row[0, :NOUT] = lin3b
    # lin2 bias in column-chunk layout [128, 32]
    b2cols = np.zeros((128, 32), np.float32)
    b2cols.T.flat[:4000] = lin2b

    L1S = 2000 // NCORES   # 250 lin1 rows per core

    per_core = []
    for c in range(NCORES):
        cb = bins[c]
        idx1 = []
        idx2 = []
        gdst = np.zeros((128, totch), np.float32)
        wtab = np.zeros((128, totch), BF)
        cnt = np.zeros((1, NCH), np.int32)
        off = 0
        for b in range(NCH):
            f1 = []
            f2 = []
            g = []
            ww = []
            for grp, nchg in ((0, cfg["ncha"][b]), (1, cfg["nchbb"][b])):
                es, ed, ew = cb[b][grp]
                nreal = len(es)
                L = nchg * 128
                assert nreal <= L
                a1 = np.zeros(L, np.int64)
                a1[:nreal] = es
                sg, row = _bininfo(a1[:nreal])
                assert (sg == grp).all()
                a2 = np.zeros(L, np.int64)
                a2[:nreal] = row
                gg = np.zeros(L, np.float32)
                gg[:nreal] = ed
                wg = np.zeros(L, np.float32)
                wg[:nreal] = ew
                f1.append(a1)
                f2.append(a2)
                g.append(gg)
                ww.append(wg)
            f1 = np.concatenate(f1)
            f2 = np.concatenate(f2)
            g = np.concatenate(g)
            ww = np.concatenate(ww)
            idx1.append(_wrap16(f1))
            idx2.append(_wrap16(f2))
            gdst[:, off:off + nchb[b]] = g.reshape(nchb[b], 128).T
            wtab[:, off:off + nchb[b]] = ww.reshape(nchb[b], 128).T.astype(BF)
            cnt[0, b] = 0
            off += nchb[b]
        idx1 = np.concatenate(idx1, axis=1)
        idx2 = np.concatenate(idx2, axis=1)

        # root features, transposed, bin-padded layout [FIN, NPAD] bf16
        xT = np.zeros((FIN, NPAD), BF)
        loc = np.arange(NPC)
        cols = (loc // BINW) * 128 + loc % BINW
        xT[:, cols] = x[c * NPC + loc].T.astype(BF)

        # lin1 shard: [1024, 256] bf16 (z layout: [max 500 pad512, mean 500
        # pad512]); rows are the contraction dim
        l1T = np.zeros((1024, 256), BF)
        sh = lin1W[c * L1S:(c + 1) * L1S].T            # [1000, 250]
        l1T[:500, :250] = sh[:500].astype(BF)
        l1T[512:1012, :250] = sh[500:].astype(BF)
        b1h = np.zeros((128, 2), np.float32)
        b1h.T.flat[:L1S] = lin1b[c * L1S:(c + 1) * L1S]

        # lin2 partial-contraction shard: rows = this core's 250 z1 entries
        # (pad 256), cols = all 4000 outputs (pad 4096)
        l2T = np.zeros((256, 4096), BF)
        l2T[:250, :4000] = lin2W[:, c * L1S:(c + 1) * L1S].T.astype(BF)

        per_core.append(dict(
            xtbl=xbf,
            idx1=idx1, idx2=idx2, gdst=gdst, wtab=wtab, cnt=cnt,
            padmask=padmask, xT=xT,
            w1relT=w1relT, w1rootT=w1rootT, w2relT=w2relT, w2rootT=w2rootT,
            b1r=b1r, b2r=b2r, p1r=p1r, p2r=p2r,
            iota128=iota128, iotaB=iotaB, ident=ident, identbf=identbf,
            ones1x128=ones1x128, ones128=ones128, onesP=onesP,
            onesPbf=onesPbf,
            l1T=l1T, b1h=b1h, l2T=l2T, b2cols=b2cols, l3T=l3T, b3row=b3row,
        ))
    return per_core


# ---------------------------------------------------------------------------
# device program
# ---------------------------------------------------------------------------

def _mid_bcast(ap, n, axis=1):
    """insert a step-0 dim of size n at position `axis` (free dims only)."""
    ap = ap.unsqueeze(axis)
    newap = list(ap.ap)
    newap[axis] = [0, n]
    return dataclasses.replace(ap, ap=newap)


def _build(cfg):
    nchb = list(cfg["nchb"])
    totch = sum(nchb)
    choff = np.concatenate([[0], np.cumsum(nchb)]).astype(int)
    NCHMAX = max(nchb)

    nc = bacc.Bacc("TRN2", target_bir_lowering=False, debug=False,
                   num_devices=NCORES)

    def din(name, shape, dt=FP32):
        return nc.dram_tensor(name, shape, dt, kind="ExternalInput")

    xtbl = din("xtbl", [N, FIN], BF16)
    idx1 = din("idx1", [128, totch * 8], I16)
    idx2 = din("idx2", [128, totch * 8], I16)
    gdst = din("gdst", [128, totch])
    wtab = din("wtab", [128, totch], BF16)
    cnt = din("cnt", [1, NCH], I32)
    padmask = din("padmask", [128, NCH])
    xT = din("xT", [FIN, NPAD], BF16)
    w1relT = din("w1relT", [FIN, HPAD], BF16)
    w1rootT = din("w1rootT", [FIN, HPAD], BF16)
    w2relT = din("w2relT", [HPAD, HPAD], BF16)
    w2rootT = din("w2rootT", [HPAD, HPAD], BF16)
    b1r = din("b1r", [128, HPAD])
    b2r = din("b2r", [128, HPAD])
    p1r = din("p1r", [128, HPAD], BF16)
    p2r = din("p2r", [128, HPAD], BF16)
    iota128 = din("iota128", [128, 128])
    iotaB = din("iotaB", [128, NBINS])
    ident = din("ident", [128, 128])
    identbf = din("identbf", [128, 128], BF16)
    ones1x128 = din("ones1x128", [1, 128])
    ones128 = din("ones128", [128, 128])
    onesP = din("onesP", [128, 1])
    onesPbf = din("onesPbf", [128, 1], BF16)
    l1T = din("l1T", [1024, 256], BF16)
    b1h = din("b1h", [128, 2])
    l2T = din("l2T", [256, 4096], BF16)
    b2cols = din("b2cols", [128, 32])
    l3T = din("l3T", [4096, 128], BF16)
    b3row = din("b3row", [1, 128])

    out = nc.dram_tensor("out", [1, NOUT], FP32, kind="ExternalOutput")

    RG = [list(range(NCORES))]

    with tile.TileContext(nc) as tc:
        with (
            tc.tile_pool(name="const", bufs=1) as cp,
            tc.tile_pool(name="gather", bufs=2) as gp,
            tc.tile_pool(name="work", bufs=1) as wp,
            tc.tile_pool(name="big", bufs=1) as bigp,
            tc.tile_pool(name="psA", bufs=2, space="PSUM") as psA,
            tc.tile_pool(name="psB", bufs=2, space="PSUM") as psB,
            tc.tile_pool(name="psS", bufs=1, space="PSUM") as psS,
            tc.tile_pool(name="dram", bufs=1, space="DRAM") as dr,
        ):
            def load(src, dt=FP32, tag=None):
                tl = cp.tile(list(src.shape), dt, tag=tag or src.name)
                nc.sync.dma_start(tl[:], src[:])
                return tl

            idx1_t = load(idx1, I16)
            idx2_t = load(idx2, I16)
            gdst_t = load(gdst)
            wtab_t = load(wtab, BF16)
            cnt_t = load(cnt, I32)
            pad_t = load(padmask)
            io_t = load(iota128)
            iob_t = load(iotaB)
            id_t = load(ident)
            idbf_t = load(identbf, BF16)
            ones_t = load(ones1x128)
            ones128_t = load(ones128)
            onesP_t = load(onesP)
            onesPbf_t = load(onesPbf, BF16)
            b1_t = load(b1r)
            b2_t = load(b2r)
            p1_t = load(p1r, BF16)
            p2_t = load(p2r, BF16)

            def load_chunks(src, nchunks, cols, tag, dt=BF16):
                ts = []
                for k in range(nchunks):
                    t = cp.tile([128, cols], dt, tag=f"{tag}{k}")
                    nc.sync.dma_start(t[:], src[k * 128:(k + 1) * 128, :cols])
                    ts.append(t)
                return ts

            w1rel_t = load_chunks(w1relT, 2, HPAD, "w1rel")
            w1root_t = load_chunks(w1rootT, 2, HPAD, "w1root")
            w2rel_t = load_chunks(w2relT, 4, HPAD, "w2rel")
            w2root_t = load_chunks(w2rootT, 4, HPAD, "w2root")
            xT_t = load_chunks(xT, 2, NPAD, "xTc")

            # DRAM internal tiles
            zsh1 = dr.tile([NPAD, 1], FP32)
            zag1 = dr.tile([NCORES * NPAD, 1], FP32, addr_space="Shared")
            zsh2 = dr.tile([NPAD, 1], FP32)
            zag2 = dr.tile([NCORES * NPAD, 1], FP32, addr_space="Shared")
            g1sh = dr.tile([NPAD, HPAD], BF16)
            g1ag = dr.tile([NCORES * NPAD, HPAD], BF16, addr_space="Shared")
            ro1in = dr.tile([2, HPAD], FP32)
            ro1ag = dr.tile([2 * NCORES, HPAD], FP32, addr_space="Shared")
            ro2in = dr.tile([2, HPAD], FP32)
            ro2ag = dr.tile([2 * NCORES, HPAD], FP32, addr_space="Shared")
            z2in = dr.tile([1, 4096], FP32)
            z2ar = dr.tile([1, 4096], FP32, addr_space="Shared")

            # per-bin edge counts: full (pads gather row 0 with w=0)
            cnt_regs = [nchb[b] * 128 for b in range(NCH)]

            # gather tiles (memset both ring buffers once: pad slots must
            # never hold NaN garbage)
            for _ in range(2):
                t1 = gp.tile([128, NCHMAX, FIN], BF16, tag="gathL1")
                t2 = gp.tile([128, NCHMAX, HPAD], BF16, tag="gathL2")
                nc.vector.memset(t1[:], 0.0)
                nc.vector.memset(t2[:], 0.0)

            # ---------------- conv layer -----------------------------------
            def conv_layer(F, pieces, idx_t, wrel_t, wroot_t, rootT_t, b_t,
                           p_t, h_all, z_all, gtag, lname):
                """per bin: gather -> weighted one-hot -> PE scatter-add ->
                transpose -> dense -> relu + score. pieces(b) yields
                (table_ap, chunk_lo, chunk_hi) gather pieces for bin b."""
                nfc = F // 128
                for b in range(NCH):
                    nch = nchb[b]
                    co = int(choff[b])
                    gt = gp.tile([128, NCHMAX, F], BF16, tag=gtag)
                    # split into <=1024-index calls (ucode-tested size)
                    for tbl_ap, c0, c1 in pieces(b):
                        for j0 in range(c0, c1, 8):
                            j1 = min(c1, j0 + 8)
                            nc.gpsimd.dma_gather(
                                gt[:, j0:j1, :], tbl_ap,
                                idx_t[:, (co + j0) * 8:(co + j1) * 8],
                                (j1 - j0) * 128, (j1 - j0) * 128, F)
                    # weighted one-hot [128, nch, 128]
                    oh = wp.tile([128, NCHMAX, 128], BF16, tag="ohw", bufs=2)
                    nc.vector.tensor_tensor(
                        out=oh[:, :nch, :],
                        in0=gdst_t[:, co:co + nch].unsqueeze(2)
                            .broadcast_to([128, nch, 128]),
                        in1=_mid_bcast(io_t[:], nch), op=OP.is_equal)
                    nc.vector.tensor_tensor(
                        out=oh[:, :nch, :], in0=oh[:, :nch, :],
                        in1=wtab_t[:, co:co + nch].unsqueeze(2)
                            .broadcast_to([128, nch, 128]), op=OP.mult)
                    # agg[dst, F] += oh.T @ gathered
                    agg_ps = psA.tile([128, HPAD], FP32, tag="aggps")
                    for c in range(nch):
                        nc.tensor.matmul(
                            out=agg_ps[:, :F], lhsT=oh[:, c, :],
                            rhs=gt[:, c, :],
                            start=(c == 0), stop=(c == nch - 1))
                    agg_sb = wp.tile([128, HPAD], BF16, tag="aggsb", bufs=2)
                    nc.vector.tensor_copy(agg_sb[:, :F], agg_ps[:, :F])
                    # transpose to [F, dst]
                    aggT = wp.tile([128, 4, 128], BF16, tag="aggT", bufs=2)
                    for fc in range(nfc):
                        tp = psB.tile([128, 128], BF16, tag="trp")
                        nc.tensor.transpose(
                            out=tp[:], in_=agg_sb[:, fc * 128:(fc + 1) * 128],
                            identity=idbf_t[:])
                        nc.vector.tensor_copy(aggT[:, fc, :], tp[:])
                    # dense: h = relu(aggT.T @ wrelT + root.T @ wrootT + b)
                    hp = psB.tile([128, HPAD], FP32, tag="hps")
                    for fc in range(nfc):
                        nc.tensor.matmul(
                            out=hp[:], lhsT=aggT[:, fc, :], rhs=wrel_t[fc][:],
                            start=(fc == 0), stop=False)
                    nroot = len(rootT_t)
                    for fc in range(nroot):
                        nc.tensor.matmul(
                            out=hp[:], lhsT=rootT_t[fc][:, b * 128:(b + 1) * 128],
                            rhs=wroot_t[fc][:], start=False,
                            stop=(fc == nroot - 1))
                    hc = h_all[:, b * HPAD:(b + 1) * HPAD]
                    nc.vector.tensor_tensor(out=hc, in0=hp[:], in1=b_t[:],
                                            op=OP.add)
                    nc.scalar.activation(hc, hc, ACT.Relu)
                    scr = wp.tile([128, HPAD], BF16, tag="scr", bufs=2)
                    nc.vector.tensor_tensor(out=scr[:], in0=hc, in1=p_t[:],
                                            op=OP.mult)
                    nc.vector.tensor_reduce(out=z_all[:, b:b + 1], in_=scr[:],
                                            op=OP.add, axis=AX.X)

            # ---------------- histogram k-th threshold ---------------------
            def topk_tau(zag, k, lname):
                """returns [128,1] tile with the k-th-largest threshold."""
                nfree = NCORES * NPAD // 128
                zt = wp.tile([128, nfree], FP32, tag="zt")
                nc.sync.dma_start(
                    zt[:], zag[:].rearrange("(p f) o -> p (f o)", p=128))
                ztb = wp.tile([128, nfree], BF16, tag="ztb")
                nc.vector.tensor_copy(ztb[:], zt[:])
                # min over real entries (pads are -1e30), max overall
                mm = wp.tile([128, 2], FP32, tag="mm")
                msk = wp.tile([128, nfree], FP32, tag="hmsk")
                nc.vector.tensor_scalar(msk[:], zt[:], -1e29, 2e30, OP.is_lt,
                                        OP.mult)
                nc.vector.tensor_tensor(out=msk[:], in0=msk[:], in1=zt[:],
                                        op=OP.add)
                nc.vector.tensor_reduce(out=mm[:, 0:1], in_=msk[:], op=OP.min,
                                        axis=AX.X)
                nc.vector.tensor_reduce(out=mm[:, 1:2], in_=zt[:], op=OP.max,
                                        axis=AX.X)
                lw = wp.tile([1, 2], FP32, tag="lw")  # [lo, w]
                mmT = wp.tile([1, 2, 128], FP32, tag="mmTs")
                for col in range(2):
                    mmT_ps = psS.tile([1, 128], FP32, tag="small")
                    nc.tensor.transpose(out=mmT_ps[:], in_=mm[:, col:col + 1],
                                        identity=id_t[:])
                    nc.vector.tensor_copy(mmT[:, col, :], mmT_ps[:])
                nc.vector.tensor_reduce(out=lw[:, 0:1], in_=mmT[:, 0, :],
                                        op=OP.min, axis=AX.X)
                nc.vector.tensor_reduce(out=lw[:, 1:2], in_=mmT[:, 1, :],
                                        op=OP.max, axis=AX.X)
                nc.vector.tensor_scalar_add(lw[:, 0:1], lw[:, 0:1], -1e-3)
                nc.vector.tensor_scalar_add(lw[:, 1:2], lw[:, 1:2], 1e-3)
                nc.vector.tensor_tensor(out=lw[:, 1:2], in0=lw[:, 1:2],
                                        in1=lw[:, 0:1], op=OP.subtract)
                nc.vector.tensor_scalar_mul(lw[:, 1:2], lw[:, 1:2], 1.0 / NBINS)

                # broadcast [lo, w] to all partitions once; all later stage
                # math is replicated on [128, *] tiles
                lwr_ps = psS.tile([128, 2], FP32, tag="small")
                nc.tensor.matmul(out=lwr_ps[:], lhsT=ones_t[:], rhs=lw[:],
                                 start=True, stop=True)
                lwr = wp.tile([128, 2], FP32, tag=f"lwr{lname}")
                nc.vector.tensor_copy(lwr[:], lwr_ps[:])

                for st in range(NSTAGES):
                    tt = wp.tile([128, NBINS], BF16, tag="tt")
                    nc.vector.tensor_scalar(tt[:], iob_t[:], lwr[:, 1:2],
                                            lwr[:, 0:1], OP.mult, OP.add)
                    # S[p, j, n] = (z[p, n] >= t[p, j]); contiguous inner
                    S = wp.tile([128, NBINS, nfree], BF16, tag="S")
                    nc.vector.tensor_tensor(
                        out=S[:], in0=_mid_bcast(ztb[:], NBINS),
                        in1=tt[:].unsqueeze(2)
                            .broadcast_to([128, NBINS, nfree]),
                        op=OP.is_ge)
                    cntp = wp.tile([128, NBINS], FP32, tag="cntp")
                    nc.vector.tensor_reduce(out=cntp[:], in_=S[:],
                                            op=OP.add, axis=AX.X)
                    # replicate totals to every partition in one matmul
                    cntr_ps = psS.tile([128, NBINS], FP32, tag="small")
                    nc.tensor.matmul(out=cntr_ps[:], lhsT=ones128_t[:],
                                     rhs=cntp[:], start=True, stop=True)
                    # fl = (count >= k), with margin for fp32r count noise
                    fl = wp.tile([128, NBINS], FP32, tag="fl")
                    js = wp.tile([128, 1], FP32, tag="js")
                    nc.vector.tensor_scalar(fl[:], cntr_ps[:], float(k) - 0.5,
                                            None, OP.is_ge)
                    nc.vector.tensor_reduce(out=js[:], in_=fl[:], op=OP.add,
                                            axis=AX.X)
                    nc.vector.tensor_scalar_add(js[:], js[:], -1.0)
                    nc.vector.tensor_scalar(lwr[:, 0:1], js[:], lwr[:, 1:2],
                                            lwr[:, 0:1], OP.mult, OP.add)
                    if st != NSTAGES - 1:
                        nc.vector.tensor_scalar_mul(lwr[:, 1:2], lwr[:, 1:2],
                                                    1.0 / NBINS)
                return lwr

            def inv_norm_b(p_t, lname):
                """[128,1] broadcast of 1/||p||."""
                sq = wp.tile([1, HPAD], FP32, tag="pnsq")
                nc.vector.tensor_tensor(out=sq[:], in0=p_t[0:1, :],
                                        in1=p_t[0:1, :], op=OP.mult)
                n2 = wp.tile([1, 1], FP32, tag="pn2")
                nc.vector.tensor_reduce(out=n2[:], in_=sq[:], op=OP.add,
                                        axis=AX.X)
                nc.scalar.activation(n2[:], n2[:], ACT.Sqrt)
                nc.vector.reciprocal(n2[:], n2[:])
                ib_ps = psS.tile([128, 1], FP32, tag="small")
                nc.tensor.matmul(out=ib_ps[:], lhsT=ones_t[:], rhs=n2[:],
                                 start=True, stop=True)
                ib = wp.tile([128, 1], FP32, tag=f"invbs{lname}")
                nc.vector.tensor_copy(ib[:], ib_ps[:])
                return ib

            # ======================= layer 1 ===============================
            h1 = bigp.tile([128, NCH * HPAD], BF16, tag="h_all")
            z1 = wp.tile([128, NCH], FP32, tag="z1")
            conv_layer(FIN, lambda b: [(xtbl[:], 0, nchb[b])],
                       idx1_t, w1rel_t, w1root_t, xT_t,
                       b1_t, p1_t, h1[:], z1[:], "gathL1", "l1")

            inv1b = inv_norm_b(p1_t, "l1")
            s1 = wp.tile([128, NCH], FP32, tag="s1")
            nc.scalar.activation(s1[:], z1[:], ACT.Tanh, scale=inv1b[:, 0:1])

            pm30 = wp.tile([128, NCH], FP32, tag="pm30")
            nc.vector.tensor_scalar(pm30[:], pad_t[:], 1.0, BIG, OP.subtract,
                                    OP.mult)
            zm1 = wp.tile([128, NCH], FP32, tag="zm1")
            nc.vector.tensor_tensor(out=zm1[:], in0=z1[:], in1=pad_t[:],
                                    op=OP.mult)
            nc.vector.tensor_tensor(out=zm1[:], in0=zm1[:], in1=pm30[:],
                                    op=OP.add)
            nc.sync.dma_start(
                zsh1[:].rearrange("(b p) o -> p (b o)", p=128), zm1[:])
            nc.gpsimd.collective_compute(
                "AllGather", OP.bypass, replica_groups=RG,
                ins=[zsh1[:]], outs=[zag1[:]])

            tau1b = topk_tau(zag1, K1, "l1")
            kp1 = wp.tile([128, NCH], FP32, tag="kp1")
            nc.vector.tensor_scalar(kp1[:], zm1[:], tau1b[:, 0:1], None,
                                    OP.is_ge)
            a1 = wp.tile([128, NCH], FP32, tag="a1")
            nc.vector.tensor_tensor(out=a1[:], in0=s1[:], in1=kp1[:],
                                    op=OP.mult)
            km30 = wp.tile([128, NCH], FP32, tag="km30")
            nc.vector.tensor_scalar(km30[:], kp1[:], 1.0, BIG, OP.subtract,
                                    OP.mult)


            # g1 (+ masked transpose) + readout 1
            gmT1 = [bigp.tile([128, NPAD], BF16, tag=f"gmT{fc}",
                              name=f"gmT1_{fc}")
                    for fc in range(4)]
            ro1s_ps = psS.tile([1, HPAD], FP32, tag="rosum")
            for b in range(NCH):
                hc = h1[:, b * HPAD:(b + 1) * HPAD]
                g1c = wp.tile([128, HPAD], BF16, tag="g1c", bufs=2)
                nc.vector.tensor_scalar(g1c[:], hc, a1[:, b:b + 1], None,
                                        OP.mult)
                nc.sync.dma_start(g1sh[b * 128:(b + 1) * 128, :], g1c[:])
                nc.tensor.matmul(out=ro1s_ps[:], lhsT=onesPbf_t[:], rhs=g1c[:],
                                 start=(b == 0), stop=(b == NCH - 1))
                gmc = wp.tile([128, HPAD], BF16, tag="gmc", bufs=2)
                nc.vector.tensor_scalar(gmc[:], hc, a1[:, b:b + 1],
                                        km30[:, b:b + 1], OP.mult, OP.add)
                for fc in range(4):
                    tp = psB.tile([128, 128], BF16, tag="trp")
                    nc.tensor.transpose(out=tp[:],
                                        in_=gmc[:, fc * 128:(fc + 1) * 128],
                                        identity=idbf_t[:])
                    nc.vector.tensor_copy(gmT1[fc][:, b * 128:(b + 1) * 128],
                                          tp[:])
            nc.gpsimd.collective_compute(
                "AllGather", OP.bypass, replica_groups=RG,
                ins=[g1sh[:]], outs=[g1ag[:]])

            m1T = wp.tile([128, 4], FP32, tag="m1T")
            for fc in range(4):
                nc.vector.tensor_reduce(out=m1T[:, fc:fc + 1], in_=gmT1[fc][:],
                                        op=OP.max, axis=AX.X)
            ro1s = wp.tile([1, HPAD], FP32, tag="ro1s")
            nc.vector.tensor_copy(ro1s[:], ro1s_ps[:])
            nc.sync.dma_start(ro1in[0:1, :], ro1s[:])
            nc.sync.dma_start(
                ro1in[1:2, :].rearrange("o (c p) -> p (o c)", p=128), m1T[:])
            nc.gpsimd.collective_compute(
                "AllGather", OP.bypass, replica_groups=RG,
                ins=[ro1in[:]], outs=[ro1ag[:]])

            # ======================= layer 2 ===============================
            h2 = bigp.tile([128, NCH * HPAD], BF16, tag="h_all")
            z2 = wp.tile([128, NCH], FP32, tag="z2")
            conv_layer(HPAD, lambda b: [(g1ag[:], 0, nchb[b])],
                       idx2_t, w2rel_t, w2root_t, gmT1,
                       b2_t, p2_t, h2[:], z2[:], "gathL2", "l2")

            inv2b = inv_norm_b(p2_t, "l2")
            s2 = wp.tile([128, NCH], FP32, tag="s2")
            nc.scalar.activation(s2[:], z2[:], ACT.Tanh, scale=inv2b[:, 0:1])
            zm2 = wp.tile([128, NCH], FP32, tag="zm2")
            nc.vector.tensor_tensor(out=zm2[:], in0=z2[:], in1=kp1[:],
                                    op=OP.mult)
            nc.vector.tensor_tensor(out=zm2[:], in0=zm2[:], in1=km30[:],
                                    op=OP.add)
            nc.sync.dma_start(
                zsh2[:].rearrange("(b p) o -> p (b o)", p=128), zm2[:])
            nc.gpsimd.collective_compute(
                "AllGather", OP.bypass, replica_groups=RG,
                ins=[zsh2[:]], outs=[zag2[:]])

            tau2b = topk_tau(zag2, K2, "l2")
            kp2 = wp.tile([128, NCH], FP32, tag="kp2")
            nc.vector.tensor_scalar(kp2[:], zm2[:], tau2b[:, 0:1], None,
                                    OP.is_ge)
            a2 = wp.tile([128, NCH], FP32, tag="a2")
            nc.vector.tensor_tensor(out=a2[:], in0=s2[:], in1=kp2[:],
                                    op=OP.mult)
            km30b = wp.tile([128, NCH], FP32, tag="km30b")
            nc.vector.tensor_scalar(km30b[:], kp2[:], 1.0, BIG, OP.subtract,
                                    OP.mult)


            ro2s_ps = psS.tile([1, HPAD], FP32, tag="rosum")
            mxh = wp.tile([128, HPAD], BF16, tag="mxh")
            nc.vector.memset(mxh[:], -1e30)
            for b in range(NCH):
                hc = h2[:, b * HPAD:(b + 1) * HPAD]
                g2c = wp.tile([128, HPAD], BF16, tag="g1c", bufs=2)
                nc.vector.tensor_scalar(g2c[:], hc, a2[:, b:b + 1], None,
                                        OP.mult)
                nc.tensor.matmul(out=ro2s_ps[:], lhsT=onesPbf_t[:], rhs=g2c[:],
                                 start=(b == 0), stop=(b == NCH - 1))
                gmc = wp.tile([128, HPAD], BF16, tag="gmc", bufs=2)
                nc.vector.tensor_scalar(gmc[:], hc, a2[:, b:b + 1],
                                        km30b[:, b:b + 1], OP.mult, OP.add)
                nc.vector.tensor_tensor(out=mxh[:], in0=mxh[:], in1=gmc[:],
                                        op=OP.max)
            m2T = wp.tile([128, 4], FP32, tag="m2T")
            for fc in range(4):
                tp = psB.tile([128, 128], BF16, tag="trp")
                nc.tensor.transpose(out=tp[:],
                                    in_=mxh[:, fc * 128:(fc + 1) * 128],
                                    identity=idbf_t[:])
                nc.vector.tensor_reduce(out=m2T[:, fc:fc + 1], in_=tp[:],
                                        op=OP.max, axis=AX.X)
            ro2s = wp.tile([1, HPAD], FP32, tag="ro2s")
            nc.vector.tensor_copy(ro2s[:], ro2s_ps[:])
            nc.sync.dma_start(ro2in[0:1, :], ro2s[:])
            nc.sync.dma_start(
                ro2in[1:2, :].rearrange("o (c p) -> p (o c)", p=128), m2T[:])
            nc.gpsimd.collective_compute(
                "AllGather", OP.bypass, replica_groups=RG,
                ins=[ro2in[:]], outs=[ro2ag[:]])

            # ======================= readout combine + head ================
            def combine(roag, kdiv, mxout, mnout):
                """[16, HPAD] AG -> maxT [128,4], meanT [128,4] (transposed)."""
                sums = wp.tile([128, 2 * NCORES, 4], FP32, tag="cmb")
                nc.sync.dma_start(
                    sums[:],
                    roag[:].rearrange("r (c p) -> p (r c)", p=128))
                s_ap = sums[:].rearrange("p (s t) c -> p c t s", t=2)
                nc.vector.tensor_reduce(out=mnout[:], in_=s_ap[:, :, 0, :],
                                        op=OP.add, axis=AX.X)
                nc.vector.tensor_reduce(out=mxout[:], in_=s_ap[:, :, 1, :],
                                        op=OP.max, axis=AX.X)
                nc.vector.tensor_scalar_mul(mnout[:], mnout[:], 1.0 / kdiv)

            mx1 = wp.tile([128, 4], FP32, tag="mx1")
            mn1 = wp.tile([128, 4], FP32, tag="mn1")
            combine(ro1ag, K1, mx1, mn1)
            mx2 = wp.tile([128, 4], FP32, tag="mx2")
            mn2 = wp.tile([128, 4], FP32, tag="mn2")
            combine(ro2ag, K2, mx2, mn2)

            zT = wp.tile([128, 8], BF16, tag="zT")
            nc.vector.tensor_tensor(out=zT[:, 0:4], in0=mx1[:], in1=mx2[:],
                                    op=OP.add)
            nc.vector.tensor_tensor(out=zT[:, 4:8], in0=mn1[:], in1=mn2[:],
                                    op=OP.add)

            # lin1: z1cols [128, 2] = relu(l1T.T @ zT + b1h), row-shard
            l1_t = load_chunks(l1T, 8, 256, "l1T")
            b1h_t = load(b1h)
            z1cols = wp.tile([128, 2], BF16, tag="z1cols")
            for m in range(2):
                o1p = psS.tile([128, 1], FP32, tag="small")
                for t in range(8):
                    nc.tensor.matmul(out=o1p[:],
                                     lhsT=l1_t[t][:, m * 128:(m + 1) * 128],
                                     rhs=zT[:, t:t + 1],
                                     start=(t == 0), stop=(t == 7))
                nc.scalar.activation(z1cols[:, m:m + 1], o1p[:], ACT.Relu,
                                     bias=b1h_t[:, m:m + 1])

            # lin2 partial contraction: z2p [1, 4096] = l2T.T @ z1cols
            l2_t = load_chunks(l2T, 2, 4096, "l2Tc")
            z2p = wp.tile([1, 4096], FP32, tag="z2p")
            for s in range(8):
                o2p = psS.tile([1, 512], FP32, tag="rosum")
                for t in range(2):
                    nc.tensor.matmul(
                        out=o2p[:], lhsT=z1cols[:, t:t + 1],
                        rhs=l2_t[t][:, s * 512:(s + 1) * 512],
                        start=(t == 0), stop=(t == 1))
                nc.vector.tensor_copy(z2p[:, s * 512:(s + 1) * 512], o2p[:])
            nc.sync.dma_start(z2in[:], z2p[:])
            nc.gpsimd.collective_compute(
                "AllReduce", OP.add, replica_groups=RG,
                ins=[z2in[:]], outs=[z2ar[:]])

            # z2cols [128, 32] = relu(z2ar + b2), column-chunk layout
            b2c_t = load(b2cols)
            z2cols = wp.tile([128, 32], BF16, tag="z2cols")
            z2f = wp.tile([128, 32], FP32, tag="z2f")
            nc.sync.dma_start(
                z2f[:], z2ar[:].rearrange("o (c p) -> p (o c)", p=128))
            nc.vector.tensor_tensor(out=z2f[:], in0=z2f[:], in1=b2c_t[:],
                                    op=OP.add)
            nc.vector.tensor_scalar_max(z2cols[:], z2f[:], 0.0)

            # lin3 replicated: out [1, 100] = l3T.T @ z2cols + b3
            l3_t = load_chunks(l3T, 32, 128, "l3T")
            b3_t = load(b3row)
            o3p = psS.tile([1, 128], FP32, tag="small")
            for t in range(32):
                nc.tensor.matmul(out=o3p[:], lhsT=z2cols[:, t:t + 1],
                                 rhs=l3_t[t][:], start=(t == 0),
                                 stop=(t == 31))
            fin = wp.tile([1, 128], FP32, tag="fin")
            nc.vector.tensor_tensor(out=fin[:], in0=o3p[:], in1=b3_t[:],
                                    op=OP.add)
            nc.scalar.activation(fin[:], fin[:], ACT.Sigmoid)
            nc.sync.dma_start(out[:], fin[:, :NOUT])

    nc.compile()
    return nc


# ---------------------------------------------------------------------------
# entry point
# ---------------------------------------------------------------------------

_CACHE = {}
TRACE = False


def kernel(**inputs):
    cfg, bins = _pack(inputs["edge_src"], inputs["edge_dst"],
                      inputs["edge_weight"])
    key = (cfg["ncha"], cfg["nchbb"])
    if key not in _CACHE:
        _CACHE[key] = _build(cfg)
    nc = _CACHE[key]
    in_maps = _host_inputs(inputs, cfg, bins)
    res = bass_utils.run_bass_kernel_spmd(
        nc, in_maps, core_ids=list(range(NCORES)), trace=TRACE)
    kernel.last_results = res
    return res.results[0]["out"]


if __name__ == "__main__":
    dat = np.load("/tmp/inputs.npz")
    inputs = {k: dat[k] for k in dat.files}
    got = kernel(**inputs)
    exp = np.load("/tmp/expected.npy")
    err = np.abs(got - exp).max()
    rel = err / np.abs(exp).max()
    print("out[0,:6] =", got[0, :6])
    print("exp[0,:6] =", exp[0, :6])
    print("max abs err:", err, "rel:", rel)


# revision 28
# speedup vs baseline: 1.0743x; 1.0743x over previous
"""Trainium2 Bass kernel for nn_Net_48301202211072 (GNN message passing).

2-layer GraphConv + TopKPooling + readout + MLP head, sharded over 8
NeuronCores. v2 design:

- Nodes sharded by dst: core c owns nodes [c*1250, (c+1)*1250), split into
  10 contiguous bins of 125 dsts (each bin maps to 128 PSUM partitions).
- Aggregation: per bin, dma_gather raw bf16 source rows (one slot per
  edge, trailing -1 pads skipped via per-core runtime counts), build a
  weighted one-hot [slots, dst] on DVE (2 ops/bin), contract on the PE:
  agg[dst, F] += oh.T @ gathered, accumulated in PSUM across chunks.
- Dense, scores, topk threshold (replicated 5-stage 64-bin histogram,
  contiguous-reduce layout), masked readout: same structure as v1 but
  bf16 operands for all matmuls and gathers.
- g1 table (layer-2 gather source) and its AllGather are bf16.
- Head: lin1 row-sharded; lin2 partial-contraction + one [1,4096]
  AllReduce; lin3 replicated (no final collective).
"""
import dataclasses
import math
import sys

import ml_dtypes
import numpy as np

sys.path.insert(0, "/opt/trn_rl_repo")

import concourse.bacc as bacc  # noqa: E402
import concourse.mybir as mybir  # noqa: E402
import concourse.tile as tile  # noqa: E402
from concourse import bass_utils  # noqa: E402

FP32 = mybir.dt.float32
BF16 = mybir.dt.bfloat16
I16 = mybir.dt.int16
I32 = mybir.dt.int32
AX = mybir.AxisListType
OP = mybir.AluOpType
ACT = mybir.ActivationFunctionType
BF = ml_dtypes.bfloat16

NCORES = 8
N = 10000
FIN = 256
HID = 500
HPAD = 512
NOUT = 100
NPC = N // NCORES          # 1250 nodes per core
NCH = 10                   # bins per core
BINW = NPC // NCH          # 125 dsts per bin
NPAD = NCH * 128           # 1280 padded rows per core
NBINS = 64
NSTAGES = 3
K1 = N // 2
K2 = N // 4
BIG = 1e30


def _bininfo(node):
    """global node id -> (source group 0/1, row in the full AG table)."""
    c = node // NPC
    d = node % NPC
    b = d // BINW
    g = (b >= NCH // 2).astype(np.int64)
    return g, c * NPAD + b * 128 + d % BINW


# ---------------------------------------------------------------------------
# host preprocessing
# ---------------------------------------------------------------------------

def _pack(edge_src, edge_dst, edge_weight):
    src = np.asarray(edge_src, np.int64)
    dst = np.asarray(edge_dst, np.int64)
    w = np.asarray(edge_weight, np.float32)

    bins = []  # [core][bin] -> (src_ids, dst_in_bin, w)
    for c in range(NCORES):
        lo = c * NPC
        m = (dst >= lo) & (dst < lo + NPC)
        es, ed, ew = src[m], dst[m] - lo, w[m]
        cb = []
        for b in range(NCH):
            mb = (ed >= b * BINW) & (ed < (b + 1) * BINW)
            cb.append((es[mb], ed[mb] - b * BINW, ew[mb]))
        bins.append(cb)

    nchb = tuple(max(1, max((len(bins[c][b][0]) + 127) // 128
                            for c in range(NCORES)))
                 for b in range(NCH))
    return dict(nchb=nchb), bins


def _wrap16(flat):
    """[L] int array (L % 128 == 0) -> [128, L//16] int16 wrapped-16."""
    L = len(flat)
    t = np.asarray(flat, np.int16).reshape(L // 16, 16).T  # [16, L//16]
    return np.ascontiguousarray(np.tile(t, (8, 1)))


def _host_inputs(inputs, cfg, bins):
    nchb = cfg["nchb"]
    totch = sum(nchb)
    x = np.asarray(inputs["x"], np.float32)
    xbf = x.astype(BF)

    def padT(a, rows, cols, dt=BF):
        out = np.zeros((rows, cols), dt)
        t = np.asarray(a, np.float32).T
        out[: t.shape[0], : t.shape[1]] = t.astype(dt)
        return out

    w1relT = padT(inputs["W1_rel"], FIN, HPAD)
    w1rootT = padT(inputs["W1_root"], FIN, HPAD)
    w2relT = padT(inputs["W2_rel"], HPAD, HPAD)
    w2rootT = padT(inputs["W2_root"], HPAD, HPAD)

    def repl(v, cols):
        out = np.zeros((128, cols), np.float32)
        vv = np.asarray(v, np.float32)
        out[:, : vv.shape[0]] = vv[None, :]
        return out

    b1r = repl(inputs["b1"], HPAD)
    b2r = repl(inputs["b2"], HPAD)
    p1r = repl(inputs["p1_w"], HPAD)
    p2r = repl(inputs["p2_w"], HPAD)

    iota128 = np.tile(np.arange(128, dtype=np.float32)[None, :], (128, 1))
    iotaB = np.tile(np.arange(NBINS, dtype=np.float32)[None, :], (128, 1))
    ident = np.eye(128, dtype=np.float32)
    identbf = np.eye(128, dtype=BF)
    ones1x128 = np.ones((1, 128), np.float32)
    ones128 = np.ones((128, 128), np.float32)
    onesP = np.ones((128, 1), np.float32)
    onesPbf = np.ones((128, 1), BF)

    # padmask [128, NCH]: partition p of any bin is real iff p < BINW
    padmask = np.zeros((128, NCH), np.float32)
    padmask[:BINW, :] = 1.0

    lin1W = np.asarray(inputs["lin1_W"], np.float32)   # [2000, 1000]
    lin2W = np.asarray(inputs["lin2_W"], np.float32)   # [4000, 2000]
    lin3W = np.asarray(inputs["lin3_W"], np.float32)   # [100, 4000]
    lin1b = np.asarray(inputs["lin1_b"], np.float32)
    lin2b = np.asarray(inputs["lin2_b"], np.float32)
    lin3b = np.asarray(inputs["lin3_b"], np.float32)

    # lin3: replicated [4096, 100] bf16
    l3T = np.zeros((4096, 128), BF)
    l3T[:4000, :NOUT] = lin3W.T.astype(BF)
    b3row = np.zeros((1, 128), np.float32)
    b3row[0, :NOUT] = lin3b
    # lin2 bias in column-chunk layout [128, 32]
    b2cols = np.zeros((128, 32), np.float32)
    b2cols.T.flat[:4000] = lin2b

    L1S = 2000 // NCORES   # 250 lin1 rows per core

    per_core = []
    for c in range(NCORES):
        cb = bins[c]
        idx1 = []
        idx2 = []
        gdst = np.zeros((128, totch), np.float32)
        wtab = np.zeros((128, totch), BF)
        cnt = np.zeros((1, NCH), np.int32)
        off = 0
        for b in range(NCH):
            es, ed, ew = cb[b]
            nreal = len(es)
            L = nchb[b] * 128
            assert 1 <= nreal <= L
            f1 = np.zeros(L, np.int64)
            f1[:nreal] = es
            f2 = np.zeros(L, np.int64)
            f2[:nreal] = _bininfo(es.astype(np.int64))[1]
            g = np.zeros(L, np.float32)
            g[:nreal] = ed
            ww = np.zeros(L, np.float32)
            ww[:nreal] = ew
            idx1.append(_wrap16(f1))
            idx2.append(_wrap16(f2))
            gdst[:, off:off + nchb[b]] = g.reshape(nchb[b], 128).T
            wtab[:, off:off + nchb[b]] = ww.reshape(nchb[b], 128).T.astype(BF)
            cnt[0, b] = 0
            off += nchb[b]
        idx1 = np.concatenate(idx1, axis=1)
        idx2 = np.concatenate(idx2, axis=1)

        # root features, transposed, bin-padded layout [FIN, NPAD] bf16
        xT = np.zeros((FIN, NPAD), BF)
        loc = np.arange(NPC)
        cols = (loc // BINW) * 128 + loc % BINW
        xT[:, cols] = x[c * NPC + loc].T.astype(BF)

        # lin1 shard: [1024, 256] bf16 (z layout: [max 500 pad512, mean 500
        # pad512]); rows are the contraction dim
        l1T = np.zeros((1024, 256), BF)
        sh = lin1W[c * L1S:(c + 1) * L1S].T            # [1000, 250]
        l1T[:500, :250] = sh[:500].astype(BF)
        l1T[512:1012, :250] = sh[500:].astype(BF)
        b1h = np.zeros((128, 2), np.float32)
        b1h.T.flat[:L1S] = lin1b[c * L1S:(c + 1) * L1S]

        # lin2 partial-contraction shard: rows = this core's 250 z1 entries
        # (pad 256), cols = all 4000 outputs (pad 4096)
        l2T = np.zeros((256, 4096), BF)
        l2T[:250, :4000] = lin2W[:, c * L1S:(c + 1) * L1S].T.astype(BF)

        per_core.append(dict(
            xtbl=xbf,
            idx1=idx1, idx2=idx2, gdst=gdst, wtab=wtab, cnt=cnt,
            padmask=padmask, xT=xT,
            w1relT=w1relT, w1rootT=w1rootT, w2relT=w2relT, w2rootT=w2rootT,
            b1r=b1r, b2r=b2r, p1r=p1r, p2r=p2r,
            iota128=iota128, iotaB=iotaB, ident=ident, identbf=identbf,
            ones1x128=ones1x128, ones128=ones128, onesP=onesP,
            onesPbf=onesPbf,
            l1T=l1T, b1h=b1h, l2T=l2T, b2cols=b2cols, l3T=l3T, b3row=b3row,
        ))
    return per_core


# ---------------------------------------------------------------------------
# device program
# ---------------------------------------------------------------------------

def _mid_bcast(ap, n, axis=1):
    """insert a step-0 dim of size n at position `axis` (free dims only)."""
    ap = ap.unsqueeze(axis)
    newap = list(ap.ap)
    newap[axis] = [0, n]
    return dataclasses.replace(ap, ap=newap)


def _build(cfg):
    nchb = list(cfg["nchb"])
    totch = sum(nchb)
    choff = np.concatenate([[0], np.cumsum(nchb)]).astype(int)
    NCHMAX = max(nchb)

    nc = bacc.Bacc("TRN2", target_bir_lowering=False, debug=False,
                   num_devices=NCORES)

    def din(name, shape, dt=FP32):
        return nc.dram_tensor(name, shape, dt, kind="ExternalInput")

    xtbl = din("xtbl", [N, FIN], BF16)
    idx1 = din("idx1", [128, totch * 8], I16)
    idx2 = din("idx2", [128, totch * 8], I16)
    gdst = din("gdst", [128, totch])
    wtab = din("wtab", [128, totch], BF16)
    cnt = din("cnt", [1, NCH], I32)
    padmask = din("padmask", [128, NCH])
    xT = din("xT", [FIN, NPAD], BF16)
    w1relT = din("w1relT", [FIN, HPAD], BF16)
    w1rootT = din("w1rootT", [FIN, HPAD], BF16)
    w2relT = din("w2relT", [HPAD, HPAD], BF16)
    w2rootT = din("w2rootT", [HPAD, HPAD], BF16)
    b1r = din("b1r", [128, HPAD])
    b2r = din("b2r", [128, HPAD])
    p1r = din("p1r", [128, HPAD])
    p2r = din("p2r", [128, HPAD])
    iota128 = din("iota128", [128, 128])
    iotaB = din("iotaB", [128, NBINS])
    ident = din("ident", [128, 128])
    identbf = din("identbf", [128, 128], BF16)
    ones1x128 = din("ones1x128", [1, 128])
    ones128 = din("ones128", [128, 128])
    onesP = din("onesP", [128, 1])
    onesPbf = din("onesPbf", [128, 1], BF16)
    l1T = din("l1T", [1024, 256], BF16)
    b1h = din("b1h", [128, 2])
    l2T = din("l2T", [256, 4096], BF16)
    b2cols = din("b2cols", [128, 32])
    l3T = din("l3T", [4096, 128], BF16)
    b3row = din("b3row", [1, 128])

    out = nc.dram_tensor("out", [1, NOUT], FP32, kind="ExternalOutput")

    RG = [list(range(NCORES))]

    with tile.TileContext(nc) as tc:
        with (
            tc.tile_pool(name="const", bufs=1) as cp,
            tc.tile_pool(name="gather", bufs=2) as gp,
            tc.tile_pool(name="work", bufs=1) as wp,
            tc.tile_pool(name="big", bufs=1) as bigp,
            tc.tile_pool(name="psA", bufs=2, space="PSUM") as psA,
            tc.tile_pool(name="psB", bufs=2, space="PSUM") as psB,
            tc.tile_pool(name="psS", bufs=1, space="PSUM") as psS,
            tc.tile_pool(name="dram", bufs=1, space="DRAM") as dr,
        ):
            def load(src, dt=FP32, tag=None):
                tl = cp.tile(list(src.shape), dt, tag=tag or src.name)
                nc.sync.dma_start(tl[:], src[:])
                return tl

            idx1_t = load(idx1, I16)
            idx2_t = load(idx2, I16)
            gdst_t = load(gdst)
            wtab_t = load(wtab, BF16)
            cnt_t = load(cnt, I32)
            pad_t = load(padmask)
            io_t = load(iota128)
            iob_t = load(iotaB)
            id_t = load(ident)
            idbf_t = load(identbf, BF16)
            ones_t = load(ones1x128)
            ones128_t = load(ones128)
            onesP_t = load(onesP)
            onesPbf_t = load(onesPbf, BF16)
            b1_t = load(b1r)
            b2_t = load(b2r)
            p1_t = load(p1r)
            p2_t = load(p2r)

            def load_chunks(src, nchunks, cols, tag, dt=BF16):
                ts = []
                for k in range(nchunks):
                    t = cp.tile([128, cols], dt, tag=f"{tag}{k}")
                    nc.sync.dma_start(t[:], src[k * 128:(k + 1) * 128, :cols])
                    ts.append(t)
                return ts

            w1rel_t = load_chunks(w1relT, 2, HPAD, "w1rel")
            w1root_t = load_chunks(w1rootT, 2, HPAD, "w1root")
            w2rel_t = load_chunks(w2relT, 4, HPAD, "w2rel")
            w2root_t = load_chunks(w2rootT, 4, HPAD, "w2root")
            xT_t = load_chunks(xT, 2, NPAD, "xTc")

            # DRAM internal tiles
            zsh1 = dr.tile([NPAD, 1], FP32)
            zag1 = dr.tile([NCORES * NPAD, 1], FP32, addr_space="Shared")
            zsh2 = dr.tile([NPAD, 1], FP32)
            zag2 = dr.tile([NCORES * NPAD, 1], FP32, addr_space="Shared")
            g1sh = dr.tile([NPAD, HPAD], BF16)
            g1ag = dr.tile([NCORES * NPAD, HPAD], BF16, addr_space="Shared")
            ro1in = dr.tile([2, HPAD], FP32)
            ro1ag = dr.tile([2 * NCORES, HPAD], FP32, addr_space="Shared")
            ro2in = dr.tile([2, HPAD], FP32)
            ro2ag = dr.tile([2 * NCORES, HPAD], FP32, addr_space="Shared")
            z2in = dr.tile([1, 4096], FP32)
            z2ar = dr.tile([1, 4096], FP32, addr_space="Shared")

            # per-bin edge counts: full (pads gather row 0 with w=0)
            cnt_regs = [nchb[b] * 128 for b in range(NCH)]

            # gather tiles (memset both ring buffers once: pad slots must
            # never hold NaN garbage)
            for _ in range(2):
                t1 = gp.tile([128, NCHMAX, FIN], BF16, tag="gathL1")
                t2 = gp.tile([128, NCHMAX, HPAD], BF16, tag="gathL2")
                nc.vector.memset(t1[:], 0.0)
                nc.vector.memset(t2[:], 0.0)

            # ---------------- conv layer -----------------------------------
            def conv_layer(F, pieces, idx_t, wrel_t, wroot_t, rootT_t, b_t,
                           p_t, h_all, z_all, gtag, lname):
                """per bin: gather -> weighted one-hot -> PE scatter-add ->
                transpose -> dense -> relu + score. pieces(b) yields
                (table_ap, chunk_lo, chunk_hi) gather pieces for bin b."""
                nfc = F // 128
                for b in range(NCH):
                    nch = nchb[b]
                    co = int(choff[b])
                    gt = gp.tile([128, NCHMAX, F], BF16, tag=gtag)
                    # split into <=1024-index calls (ucode-tested size)
                    for tbl_ap, c0, c1 in pieces(b):
                        for j0 in range(c0, c1, 8):
                            j1 = min(c1, j0 + 8)
                            nc.gpsimd.dma_gather(
                                gt[:, j0:j1, :], tbl_ap,
                                idx_t[:, (co + j0) * 8:(co + j1) * 8],
                                (j1 - j0) * 128, (j1 - j0) * 128, F)
                    # weighted one-hot [128, nch, 128]
                    oh = wp.tile([128, NCHMAX, 128], BF16, tag="ohw", bufs=2)
                    nc.vector.tensor_tensor(
                        out=oh[:, :nch, :],
                        in0=gdst_t[:, co:co + nch].unsqueeze(2)
                            .broadcast_to([128, nch, 128]),
                        in1=_mid_bcast(io_t[:], nch), op=OP.is_equal)
                    nc.vector.tensor_tensor(
                        out=oh[:, :nch, :], in0=oh[:, :nch, :],
                        in1=wtab_t[:, co:co + nch].unsqueeze(2)
                            .broadcast_to([128, nch, 128]), op=OP.mult)
                    # agg[dst, F] += oh.T @ gathered
                    agg_ps = psA.tile([128, HPAD], FP32, tag="aggps")
                    for c in range(nch):
                        nc.tensor.matmul(
                            out=agg_ps[:, :F], lhsT=oh[:, c, :],
                            rhs=gt[:, c, :],
                            start=(c == 0), stop=(c == nch - 1))
                    agg_sb = wp.tile([128, HPAD], BF16, tag="aggsb", bufs=2)
                    nc.vector.tensor_copy(agg_sb[:, :F], agg_ps[:, :F])
                    # transpose to [F, dst]
                    aggT = wp.tile([128, 4, 128], BF16, tag="aggT", bufs=2)
                    for fc in range(nfc):
                        tp = psB.tile([128, 128], BF16, tag="trp")
                        nc.tensor.transpose(
                            out=tp[:], in_=agg_sb[:, fc * 128:(fc + 1) * 128],
                            identity=idbf_t[:])
                        nc.vector.tensor_copy(aggT[:, fc, :], tp[:])
                    # dense: h = relu(aggT.T @ wrelT + root.T @ wrootT + b)
                    hp = psB.tile([128, HPAD], FP32, tag="hps")
                    for fc in range(nfc):
                        nc.tensor.matmul(
                            out=hp[:], lhsT=aggT[:, fc, :], rhs=wrel_t[fc][:],
                            start=(fc == 0), stop=False)
                    nroot = len(rootT_t)
                    for fc in range(nroot):
                        nc.tensor.matmul(
                            out=hp[:], lhsT=rootT_t[fc][:, b * 128:(b + 1) * 128],
                            rhs=wroot_t[fc][:], start=False,
                            stop=(fc == nroot - 1))
                    hc = h_all[:, b * HPAD:(b + 1) * HPAD]
                    nc.vector.tensor_tensor(out=hc, in0=hp[:], in1=b_t[:],
                                            op=OP.add)
                    nc.scalar.activation(hc, hc, ACT.Relu)
                    scr = wp.tile([128, HPAD], FP32, tag="scr", bufs=2)
                    nc.vector.tensor_tensor(out=scr[:], in0=hc, in1=p_t[:],
                                            op=OP.mult)
                    nc.vector.tensor_reduce(out=z_all[:, b:b + 1], in_=scr[:],
                                            op=OP.add, axis=AX.X)

            # ---------------- histogram k-th threshold ---------------------
            def topk_tau(zag, k, lname):
                """returns [128,1] tile with the k-th-largest threshold."""
                nfree = NCORES * NPAD // 128
                zt = wp.tile([128, nfree], FP32, tag="zt")
                nc.sync.dma_start(
                    zt[:], zag[:].rearrange("(p f) o -> p (f o)", p=128))
                ztb = wp.tile([128, nfree], BF16, tag="ztb")
                nc.vector.tensor_copy(ztb[:], zt[:])
                # min over real entries (pads are -1e30), max overall
                mm = wp.tile([128, 2], FP32, tag="mm")
                msk = wp.tile([128, nfree], FP32, tag="hmsk")
                nc.vector.tensor_scalar(msk[:], zt[:], -1e29, 2e30, OP.is_lt,
                                        OP.mult)
                nc.vector.tensor_tensor(out=msk[:], in0=msk[:], in1=zt[:],
                                        op=OP.add)
                nc.vector.tensor_reduce(out=mm[:, 0:1], in_=msk[:], op=OP.min,
                                        axis=AX.X)
                nc.vector.tensor_reduce(out=mm[:, 1:2], in_=zt[:], op=OP.max,
                                        axis=AX.X)
                lw = wp.tile([1, 2], FP32, tag="lw")  # [lo, w]
                mmT = wp.tile([1, 2, 128], FP32, tag="mmTs")
                for col in range(2):
                    mmT_ps = psS.tile([1, 128], FP32, tag="small")
                    nc.tensor.transpose(out=mmT_ps[:], in_=mm[:, col:col + 1],
                                        identity=id_t[:])
                    nc.vector.tensor_copy(mmT[:, col, :], mmT_ps[:])
                nc.vector.tensor_reduce(out=lw[:, 0:1], in_=mmT[:, 0, :],
                                        op=OP.min, axis=AX.X)
                nc.vector.tensor_reduce(out=lw[:, 1:2], in_=mmT[:, 1, :],
                                        op=OP.max, axis=AX.X)
                nc.vector.tensor_scalar_add(lw[:, 0:1], lw[:, 0:1], -1e-3)
                nc.vector.tensor_scalar_add(lw[:, 1:2], lw[:, 1:2], 1e-3)
                nc.vector.tensor_tensor(out=lw[:, 1:2], in0=lw[:, 1:2],
                                        in1=lw[:, 0:1], op=OP.subtract)
                nc.vector.tensor_scalar_mul(lw[:, 1:2], lw[:, 1:2], 1.0 / NBINS)

                # broadcast [lo, w] to all partitions once; all later stage
                # math is replicated on [128, *] tiles
                lwr_ps = psS.tile([128, 2], FP32, tag="small")
                nc.tensor.matmul(out=lwr_ps[:], lhsT=ones_t[:], rhs=lw[:],
                                 start=True, stop=True)
                lwr = wp.tile([128, 2], FP32, tag=f"lwr{lname}")
                nc.vector.tensor_copy(lwr[:], lwr_ps[:])

                for st in range(NSTAGES):
                    tt = wp.tile([128, NBINS], BF16, tag="tt")
                    nc.vector.tensor_scalar(tt[:], iob_t[:], lwr[:, 1:2],
                                            lwr[:, 0:1], OP.mult, OP.add)
                    # S[p, j, n] = (z[p, n] >= t[p, j]); contiguous inner
                    S = wp.tile([128, NBINS, nfree], BF16, tag="S")
                    nc.vector.tensor_tensor(
                        out=S[:], in0=_mid_bcast(ztb[:], NBINS),
                        in1=tt[:].unsqueeze(2)
                            .broadcast_to([128, NBINS, nfree]),
                        op=OP.is_ge)
                    cntp = wp.tile([128, NBINS], FP32, tag="cntp")
                    nc.vector.tensor_reduce(out=cntp[:], in_=S[:],
                                            op=OP.add, axis=AX.X)
                    # replicate totals to every partition in one matmul
                    cntr_ps = psS.tile([128, NBINS], FP32, tag="small")
                    nc.tensor.matmul(out=cntr_ps[:], lhsT=ones128_t[:],
                                     rhs=cntp[:], start=True, stop=True)
                    # fl = (count >= k), with margin for fp32r count noise
                    fl = wp.tile([128, NBINS], FP32, tag="fl")
                    js = wp.tile([128, 1], FP32, tag="js")
                    nc.vector.tensor_scalar(fl[:], cntr_ps[:], float(k) - 0.5,
                                            None, OP.is_ge)
                    nc.vector.tensor_reduce(out=js[:], in_=fl[:], op=OP.add,
                                            axis=AX.X)
                    nc.vector.tensor_scalar_add(js[:], js[:], -1.0)
                    nc.vector.tensor_scalar(lwr[:, 0:1], js[:], lwr[:, 1:2],
                                            lwr[:, 0:1], OP.mult, OP.add)
                    if st != NSTAGES - 1:
                        nc.vector.tensor_scalar_mul(lwr[:, 1:2], lwr[:, 1:2],
                                                    1.0 / NBINS)
                return lwr

            def inv_norm_b(p_t, lname):
                """[128,1] broadcast of 1/||p||."""
                sq = wp.tile([1, HPAD], FP32, tag="pnsq")
                nc.vector.tensor_tensor(out=sq[:], in0=p_t[0:1, :],
                                        in1=p_t[0:1, :], op=OP.mult)
                n2 = wp.tile([1, 1], FP32, tag="pn2")
                nc.vector.tensor_reduce(out=n2[:], in_=sq[:], op=OP.add,
                                        axis=AX.X)
                nc.scalar.activation(n2[:], n2[:], ACT.Sqrt)
                nc.vector.reciprocal(n2[:], n2[:])
                ib_ps = psS.tile([128, 1], FP32, tag="small")
                nc.tensor.matmul(out=ib_ps[:], lhsT=ones_t[:], rhs=n2[:],
                                 start=True, stop=True)
                ib = wp.tile([128, 1], FP32, tag=f"invbs{lname}")
                nc.vector.tensor_copy(ib[:], ib_ps[:])
                return ib

            # ======================= layer 1 ===============================
            h1 = bigp.tile([128, NCH * HPAD], FP32, tag="h_all")
            z1 = wp.tile([128, NCH], FP32, tag="z1")
            conv_layer(FIN, lambda b: [(xtbl[:], 0, nchb[b])],
                       idx1_t, w1rel_t, w1root_t, xT_t,
                       b1_t, p1_t, h1[:], z1[:], "gathL1", "l1")

            inv1b = inv_norm_b(p1_t, "l1")
            s1 = wp.tile([128, NCH], FP32, tag="s1")
            nc.scalar.activation(s1[:], z1[:], ACT.Tanh, scale=inv1b[:, 0:1])

            pm30 = wp.tile([128, NCH], FP32, tag="pm30")
            nc.vector.tensor_scalar(pm30[:], pad_t[:], 1.0, BIG, OP.subtract,
                                    OP.mult)
            zm1 = wp.tile([128, NCH], FP32, tag="zm1")
            nc.vector.tensor_tensor(out=zm1[:], in0=z1[:], in1=pad_t[:],
                                    op=OP.mult)
            nc.vector.tensor_tensor(out=zm1[:], in0=zm1[:], in1=pm30[:],
                                    op=OP.add)
            nc.sync.dma_start(
                zsh1[:].rearrange("(b p) o -> p (b o)", p=128), zm1[:])
            nc.gpsimd.collective_compute(
                "AllGather", OP.bypass, replica_groups=RG,
                ins=[zsh1[:]], outs=[zag1[:]])

            tau1b = topk_tau(zag1, K1, "l1")
            kp1 = wp.tile([128, NCH], FP32, tag="kp1")
            nc.vector.tensor_scalar(kp1[:], zm1[:], tau1b[:, 0:1], None,
                                    OP.is_ge)
            a1 = wp.tile([128, NCH], FP32, tag="a1")
            nc.vector.tensor_tensor(out=a1[:], in0=s1[:], in1=kp1[:],
                                    op=OP.mult)
            km30 = wp.tile([128, NCH], FP32, tag="km30")
            nc.vector.tensor_scalar(km30[:], kp1[:], 1.0, BIG, OP.subtract,
                                    OP.mult)

            # g1 (+ masked transpose) + readout 1
            gmT1 = [bigp.tile([128, NPAD], BF16, tag=f"gmT{fc}",
                              name=f"gmT1_{fc}")
                    for fc in range(4)]
            ro1s_ps = psS.tile([1, HPAD], FP32, tag="rosum")
            for b in range(NCH):
                hc = h1[:, b * HPAD:(b + 1) * HPAD]
                g1c = wp.tile([128, HPAD], BF16, tag="g1c", bufs=2)
                nc.vector.tensor_scalar(g1c[:], hc, a1[:, b:b + 1], None,
                                        OP.mult)
                nc.sync.dma_start(g1sh[b * 128:(b + 1) * 128, :], g1c[:])
                nc.tensor.matmul(out=ro1s_ps[:], lhsT=onesPbf_t[:], rhs=g1c[:],
                                 start=(b == 0), stop=(b == NCH - 1))
                gmc = wp.tile([128, HPAD], BF16, tag="gmc", bufs=2)
                nc.vector.tensor_scalar(gmc[:], hc, a1[:, b:b + 1],
                                        km30[:, b:b + 1], OP.mult, OP.add)
                for fc in range(4):
                    tp = psB.tile([128, 128], BF16, tag="trp")
                    nc.tensor.transpose(out=tp[:],
                                        in_=gmc[:, fc * 128:(fc + 1) * 128],
                                        identity=idbf_t[:])
                    nc.vector.tensor_copy(gmT1[fc][:, b * 128:(b + 1) * 128],
                                          tp[:])
            nc.gpsimd.collective_compute(
                "AllGather", OP.bypass, replica_groups=RG,
                ins=[g1sh[:]], outs=[g1ag[:]])

            m1T = wp.tile([128, 4], FP32, tag="m1T")
            for fc in range(4):
                nc.vector.tensor_reduce(out=m1T[:, fc:fc + 1], in_=gmT1[fc][:],
                                        op=OP.max, axis=AX.X)
            ro1s = wp.tile([1, HPAD], FP32, tag="ro1s")
            nc.vector.tensor_copy(ro1s[:], ro1s_ps[:])
            nc.sync.dma_start(ro1in[0:1, :], ro1s[:])
            nc.sync.dma_start(
                ro1in[1:2, :].rearrange("o (c p) -> p (o c)", p=128), m1T[:])
            nc.gpsimd.collective_compute(
                "AllGather", OP.bypass, replica_groups=RG,
                ins=[ro1in[:]], outs=[ro1ag[:]])

            # ======================= layer 2 ===============================
            h2 = bigp.tile([128, NCH * HPAD], FP32, tag="h_all")
            z2 = wp.tile([128, NCH], FP32, tag="z2")
            conv_layer(HPAD, lambda b: [(g1ag[:], 0, nchb[b])],
                       idx2_t, w2rel_t, w2root_t, gmT1,
                       b2_t, p2_t, h2[:], z2[:], "gathL2", "l2")

            inv2b = inv_norm_b(p2_t, "l2")
            s2 = wp.tile([128, NCH], FP32, tag="s2")
            nc.scalar.activation(s2[:], z2[:], ACT.Tanh, scale=inv2b[:, 0:1])
            zm2 = wp.tile([128, NCH], FP32, tag="zm2")
            nc.vector.tensor_tensor(out=zm2[:], in0=z2[:], in1=kp1[:],
                                    op=OP.mult)
            nc.vector.tensor_tensor(out=zm2[:], in0=zm2[:], in1=km30[:],
                                    op=OP.add)
            nc.sync.dma_start(
                zsh2[:].rearrange("(b p) o -> p (b o)", p=128), zm2[:])
            nc.gpsimd.collective_compute(
                "AllGather", OP.bypass, replica_groups=RG,
                ins=[zsh2[:]], outs=[zag2[:]])

            tau2b = topk_tau(zag2, K2, "l2")
            kp2 = wp.tile([128, NCH], FP32, tag="kp2")
            nc.vector.tensor_scalar(kp2[:], zm2[:], tau2b[:, 0:1], None,
                                    OP.is_ge)
            a2 = wp.tile([128, NCH], FP32, tag="a2")
            nc.vector.tensor_tensor(out=a2[:], in0=s2[:], in1=kp2[:],
                                    op=OP.mult)
            km30b = wp.tile([128, NCH], FP32, tag="km30b")
            nc.vector.tensor_scalar(km30b[:], kp2[:], 1.0, BIG, OP.subtract,
                                    OP.mult)

            ro2s_ps = psS.tile([1, HPAD], FP32, tag="rosum")
            m2T = wp.tile([128, 4], FP32, tag="m2T")
            nc.vector.memset(m2T[:], -1e30)
            for b in range(NCH):
                hc = h2[:, b * HPAD:(b + 1) * HPAD]
                g2c = wp.tile([128, HPAD], BF16, tag="g1c", bufs=2)
                nc.vector.tensor_scalar(g2c[:], hc, a2[:, b:b + 1], None,
                                        OP.mult)
                nc.tensor.matmul(out=ro2s_ps[:], lhsT=onesPbf_t[:], rhs=g2c[:],
                                 start=(b == 0), stop=(b == NCH - 1))
                gmc = wp.tile([128, HPAD], BF16, tag="gmc", bufs=2)
                nc.vector.tensor_scalar(gmc[:], hc, a2[:, b:b + 1],
                                        km30b[:, b:b + 1], OP.mult, OP.add)
                for fc in range(4):
                    tp = psB.tile([128, 128], BF16, tag="trp")
                    nc.tensor.transpose(out=tp[:],
                                        in_=gmc[:, fc * 128:(fc + 1) * 128],
                                        identity=idbf_t[:])
                    red = wp.tile([128, 1], FP32, tag="redm", bufs=2)
                    nc.vector.tensor_reduce(out=red[:], in_=tp[:], op=OP.max,
                                            axis=AX.X)
                    nc.vector.tensor_tensor(out=m2T[:, fc:fc + 1],
                                            in0=m2T[:, fc:fc + 1], in1=red[:],
                                            op=OP.max)
            ro2s = wp.tile([1, HPAD], FP32, tag="ro2s")
            nc.vector.tensor_copy(ro2s[:], ro2s_ps[:])
            nc.sync.dma_start(ro2in[0:1, :], ro2s[:])
            nc.sync.dma_start(
                ro2in[1:2, :].rearrange("o (c p) -> p (o c)", p=128), m2T[:])
            nc.gpsimd.collective_compute(
                "AllGather", OP.bypass, replica_groups=RG,
                ins=[ro2in[:]], outs=[ro2ag[:]])

            # ======================= readout combine + head ================
            def combine(roag, kdiv, mxout, mnout):
                """[16, HPAD] AG -> maxT [128,4], meanT [128,4] (transposed)."""
                sums = wp.tile([128, 2 * NCORES, 4], FP32, tag="cmb")
                nc.sync.dma_start(
                    sums[:],
                    roag[:].rearrange("r (c p) -> p (r c)", p=128))
                s_ap = sums[:].rearrange("p (s t) c -> p c t s", t=2)
                nc.vector.tensor_reduce(out=mnout[:], in_=s_ap[:, :, 0, :],
                                        op=OP.add, axis=AX.X)
                nc.vector.tensor_reduce(out=mxout[:], in_=s_ap[:, :, 1, :],
                                        op=OP.max, axis=AX.X)
                nc.vector.tensor_scalar_mul(mnout[:], mnout[:], 1.0 / kdiv)

            mx1 = wp.tile([128, 4], FP32, tag="mx1")
            mn1 = wp.tile([128, 4], FP32, tag="mn1")
            combine(ro1ag, K1, mx1, mn1)
            mx2 = wp.tile([128, 4], FP32, tag="mx2")
            mn2 = wp.tile([128, 4], FP32, tag="mn2")
            combine(ro2ag, K2, mx2, mn2)

            zT = wp.tile([128, 8], BF16, tag="zT")
            nc.vector.tensor_tensor(out=zT[:, 0:4], in0=mx1[:], in1=mx2[:],
                                    op=OP.add)
            nc.vector.tensor_tensor(out=zT[:, 4:8], in0=mn1[:], in1=mn2[:],
                                    op=OP.add)

            # lin1: z1cols [128, 2] = relu(l1T.T @ zT + b1h), row-shard
            l1_t = load_chunks(l1T, 8, 256, "l1T")
            b1h_t = load(b1h)
            z1cols = wp.tile([128, 2], BF16, tag="z1cols")
            for m in range(2):
                o1p = psS.tile([128, 1], FP32, tag="small")
                for t in range(8):
                    nc.tensor.matmul(out=o1p[:],
                                     lhsT=l1_t[t][:, m * 128:(m + 1) * 128],
                                     rhs=zT[:, t:t + 1],
                                     start=(t == 0), stop=(t == 7))
                nc.scalar.activation(z1cols[:, m:m + 1], o1p[:], ACT.Relu,
                                     bias=b1h_t[:, m:m + 1])

            # lin2 partial contraction: z2p [1, 4096] = l2T.T @ z1cols
            l2_t = load_chunks(l2T, 2, 4096, "l2Tc")
            z2p = wp.tile([1, 4096], FP32, tag="z2p")
            for s in range(8):
                o2p = psS.tile([1, 512], FP32, tag="rosum")
                for t in range(2):
                    nc.tensor.matmul(
                        out=o2p[:], lhsT=z1cols[:, t:t + 1],
                        rhs=l2_t[t][:, s * 512:(s + 1) * 512],
                        start=(t == 0), stop=(t == 1))
                nc.vector.tensor_copy(z2p[:, s * 512:(s + 1) * 512], o2p[:])
            nc.sync.dma_start(z2in[:], z2p[:])
            nc.gpsimd.collective_compute(
                "AllReduce", OP.add, replica_groups=RG,
                ins=[z2in[:]], outs=[z2ar[:]])

            # z2cols [128, 32] = relu(z2ar + b2), column-chunk layout
            b2c_t = load(b2cols)
            z2cols = wp.tile([128, 32], BF16, tag="z2cols")
            z2f = wp.tile([128, 32], FP32, tag="z2f")
            nc.sync.dma_start(
                z2f[:], z2ar[:].rearrange("o (c p) -> p (o c)", p=128))
            nc.vector.tensor_tensor(out=z2f[:], in0=z2f[:], in1=b2c_t[:],
                                    op=OP.add)
            nc.vector.tensor_scalar_max(z2cols[:], z2f[:], 0.0)

            # lin3 replicated: out [1, 100] = l3T.T @ z2cols + b3
            l3_t = load_chunks(l3T, 32, 128, "l3T")
            b3_t = load(b3row)
            o3p = psS.tile([1, 128], FP32, tag="small")
            for t in range(32):
                nc.tensor.matmul(out=o3p[:], lhsT=z2cols[:, t:t + 1],
                                 rhs=l3_t[t][:], start=(t == 0),
                                 stop=(t == 31))
            fin = wp.tile([1, 128], FP32, tag="fin")
            nc.vector.tensor_tensor(out=fin[:], in0=o3p[:], in1=b3_t[:],
                                    op=OP.add)
            nc.scalar.activation(fin[:], fin[:], ACT.Sigmoid)
            nc.sync.dma_start(out[:], fin[:, :NOUT])

    nc.compile()
    return nc


# ---------------------------------------------------------------------------
# entry point
# ---------------------------------------------------------------------------

_CACHE = {}
TRACE = False


def kernel(**inputs):
    cfg, bins = _pack(inputs["edge_src"], inputs["edge_dst"],
                      inputs["edge_weight"])
    key = cfg["nchb"]
    if key not in _CACHE:
        _CACHE[key] = _build(cfg)
    nc = _CACHE[key]
    in_maps = _host_inputs(inputs, cfg, bins)
    res = bass_utils.run_bass_kernel_spmd(
        nc, in_maps, core_ids=list(range(NCORES)), trace=TRACE)
    kernel.last_results = res
    return res.results[0]["out"]


if __name__ == "__main__":
    dat = np.load("/tmp/inputs.npz")
    inputs = {k: dat[k] for k in dat.files}
    got = kernel(**inputs)
    exp = np.load("/tmp/expected.npy")
    err = np.abs(got - exp).max()
    rel = err / np.abs(exp).max()
    print("out[0,:6] =", got[0, :6])
    print("exp[0,:6] =", exp[0, :6])
    print("max abs err:", err, "rel:", rel)


# revision 29
# speedup vs baseline: 1.0764x; 1.0020x over previous
"""Trainium2 Bass kernel for nn_Net_48301202211072 (GNN message passing).

2-layer GraphConv + TopKPooling + readout + MLP head, sharded over 8
NeuronCores. v2 design:

- Nodes sharded by dst: core c owns nodes [c*1250, (c+1)*1250), split into
  10 contiguous bins of 125 dsts (each bin maps to 128 PSUM partitions).
- Aggregation: per bin, dma_gather raw bf16 source rows (one slot per
  edge, trailing -1 pads skipped via per-core runtime counts), build a
  weighted one-hot [slots, dst] on DVE (2 ops/bin), contract on the PE:
  agg[dst, F] += oh.T @ gathered, accumulated in PSUM across chunks.
- Dense, scores, topk threshold (replicated 5-stage 64-bin histogram,
  contiguous-reduce layout), masked readout: same structure as v1 but
  bf16 operands for all matmuls and gathers.
- g1 table (layer-2 gather source) and its AllGather are bf16.
- Head: lin1 row-sharded; lin2 partial-contraction + one [1,4096]
  AllReduce; lin3 replicated (no final collective).
"""
import dataclasses
import math
import sys

import ml_dtypes
import numpy as np

sys.path.insert(0, "/opt/trn_rl_repo")

import concourse.bacc as bacc  # noqa: E402
import concourse.mybir as mybir  # noqa: E402
import concourse.tile as tile  # noqa: E402
from concourse import bass_utils  # noqa: E402

FP32 = mybir.dt.float32
BF16 = mybir.dt.bfloat16
I16 = mybir.dt.int16
I32 = mybir.dt.int32
AX = mybir.AxisListType
OP = mybir.AluOpType
ACT = mybir.ActivationFunctionType
BF = ml_dtypes.bfloat16

NCORES = 8
N = 10000
FIN = 256
HID = 500
HPAD = 512
NOUT = 100
NPC = N // NCORES          # 1250 nodes per core
NCH = 10                   # bins per core
BINW = NPC // NCH          # 125 dsts per bin
NPAD = NCH * 128           # 1280 padded rows per core
NBINS = 64
NSTAGES = 3
K1 = N // 2
K2 = N // 4
BIG = 1e30


def _bininfo(node):
    """global node id -> (source group 0/1, row in the full AG table)."""
    c = node // NPC
    d = node % NPC
    b = d // BINW
    g = (b >= NCH // 2).astype(np.int64)
    return g, c * NPAD + b * 128 + d % BINW


# ---------------------------------------------------------------------------
# host preprocessing
# ---------------------------------------------------------------------------

def _pack(edge_src, edge_dst, edge_weight):
    src = np.asarray(edge_src, np.int64)
    dst = np.asarray(edge_dst, np.int64)
    w = np.asarray(edge_weight, np.float32)

    bins = []  # [core][bin] -> (src_ids, dst_in_bin, w)
    for c in range(NCORES):
        lo = c * NPC
        m = (dst >= lo) & (dst < lo + NPC)
        es, ed, ew = src[m], dst[m] - lo, w[m]
        cb = []
        for b in range(NCH):
            mb = (ed >= b * BINW) & (ed < (b + 1) * BINW)
            cb.append((es[mb], ed[mb] - b * BINW, ew[mb]))
        bins.append(cb)

    nchb = tuple(max(1, max((len(bins[c][b][0]) + 127) // 128
                            for c in range(NCORES)))
                 for b in range(NCH))
    return dict(nchb=nchb), bins


def _wrap16(flat):
    """[L] int array (L % 128 == 0) -> [128, L//16] int16 wrapped-16."""
    L = len(flat)
    t = np.asarray(flat, np.int16).reshape(L // 16, 16).T  # [16, L//16]
    return np.ascontiguousarray(np.tile(t, (8, 1)))


def _host_inputs(inputs, cfg, bins):
    nchb = cfg["nchb"]
    totch = sum(nchb)
    x = np.asarray(inputs["x"], np.float32)
    xbf = x.astype(BF)

    def padT(a, rows, cols, dt=BF):
        out = np.zeros((rows, cols), dt)
        t = np.asarray(a, np.float32).T
        out[: t.shape[0], : t.shape[1]] = t.astype(dt)
        return out

    w1relT = padT(inputs["W1_rel"], FIN, HPAD)
    w1rootT = padT(inputs["W1_root"], FIN, HPAD)
    w2relT = padT(inputs["W2_rel"], HPAD, HPAD)
    w2rootT = padT(inputs["W2_root"], HPAD, HPAD)

    def repl(v, cols):
        out = np.zeros((128, cols), np.float32)
        vv = np.asarray(v, np.float32)
        out[:, : vv.shape[0]] = vv[None, :]
        return out

    b1r = repl(inputs["b1"], HPAD)
    b2r = repl(inputs["b2"], HPAD)
    p1r = repl(inputs["p1_w"], HPAD)
    p2r = repl(inputs["p2_w"], HPAD)

    iota128 = np.tile(np.arange(128, dtype=np.float32)[None, :], (128, 1))
    iotaB = np.tile(np.arange(NBINS, dtype=np.float32)[None, :], (128, 1))
    ident = np.eye(128, dtype=np.float32)
    identbf = np.eye(128, dtype=BF)
    ones1x128 = np.ones((1, 128), np.float32)
    ones128 = np.ones((128, 128), np.float32)
    onesP = np.ones((128, 1), np.float32)
    onesPbf = np.ones((128, 1), BF)

    # padmask [128, NCH]: partition p of any bin is real iff p < BINW
    padmask = np.zeros((128, NCH), np.float32)
    padmask[:BINW, :] = 1.0

    lin1W = np.asarray(inputs["lin1_W"], np.float32)   # [2000, 1000]
    lin2W = np.asarray(inputs["lin2_W"], np.float32)   # [4000, 2000]
    lin3W = np.asarray(inputs["lin3_W"], np.float32)   # [100, 4000]
    lin1b = np.asarray(inputs["lin1_b"], np.float32)
    lin2b = np.asarray(inputs["lin2_b"], np.float32)
    lin3b = np.asarray(inputs["lin3_b"], np.float32)

    # lin3: replicated [4096, 100] bf16
    l3T = np.zeros((4096, 128), BF)
    l3T[:4000, :NOUT] = lin3W.T.astype(BF)
    b3row = np.zeros((1, 128), np.float32)
    b3row[0, :NOUT] = lin3b
    # lin2 bias in column-chunk layout [128, 32]
    b2cols = np.zeros((128, 32), np.float32)
    b2cols.T.flat[:4000] = lin2b

    L1S = 2000 // NCORES   # 250 lin1 rows per core

    per_core = []
    for c in range(NCORES):
        cb = bins[c]
        idx1 = []
        idx2 = []
        gdst = np.zeros((128, totch), np.float32)
        wtab = np.zeros((128, totch), BF)
        cnt = np.zeros((1, NCH), np.int32)
        off = 0
        for b in range(NCH):
            es, ed, ew = cb[b]
            nreal = len(es)
            L = nchb[b] * 128
            assert 1 <= nreal <= L
            f1 = np.zeros(L, np.int64)
            f1[:nreal] = es
            f2 = np.zeros(L, np.int64)
            f2[:nreal] = _bininfo(es.astype(np.int64))[1]
            g = np.zeros(L, np.float32)
            g[:nreal] = ed
            ww = np.zeros(L, np.float32)
            ww[:nreal] = ew
            idx1.append(_wrap16(f1))
            idx2.append(_wrap16(f2))
            gdst[:, off:off + nchb[b]] = g.reshape(nchb[b], 128).T
            wtab[:, off:off + nchb[b]] = ww.reshape(nchb[b], 128).T.astype(BF)
            cnt[0, b] = 0
            off += nchb[b]
        idx1 = np.concatenate(idx1, axis=1)
        idx2 = np.concatenate(idx2, axis=1)

        # root features, transposed, bin-padded layout [FIN, NPAD] bf16
        xT = np.zeros((FIN, NPAD), BF)
        loc = np.arange(NPC)
        cols = (loc // BINW) * 128 + loc % BINW
        xT[:, cols] = x[c * NPC + loc].T.astype(BF)

        # lin1 shard: [1024, 256] bf16 (z layout: [max 500 pad512, mean 500
        # pad512]); rows are the contraction dim
        l1T = np.zeros((1024, 256), BF)
        sh = lin1W[c * L1S:(c + 1) * L1S].T            # [1000, 250]
        l1T[:500, :250] = sh[:500].astype(BF)
        l1T[512:1012, :250] = sh[500:].astype(BF)
        b1h = np.zeros((128, 2), np.float32)
        b1h.T.flat[:L1S] = lin1b[c * L1S:(c + 1) * L1S]

        # lin2 partial-contraction shard: rows = this core's 250 z1 entries
        # (pad 256), cols = all 4000 outputs (pad 4096)
        l2T = np.zeros((256, 4096), BF)
        l2T[:250, :4000] = lin2W[:, c * L1S:(c + 1) * L1S].T.astype(BF)

        per_core.append(dict(
            xtbl=xbf,
            idx1=idx1, idx2=idx2, gdst=gdst, wtab=wtab, cnt=cnt,
            padmask=padmask, xT=xT,
            w1relT=w1relT, w1rootT=w1rootT, w2relT=w2relT, w2rootT=w2rootT,
            b1r=b1r, b2r=b2r, p1r=p1r, p2r=p2r,
            iota128=iota128, iotaB=iotaB, ident=ident, identbf=identbf,
            ones1x128=ones1x128, ones128=ones128, onesP=onesP,
            onesPbf=onesPbf,
            l1T=l1T, b1h=b1h, l2T=l2T, b2cols=b2cols, l3T=l3T, b3row=b3row,
        ))
    return per_core


# ---------------------------------------------------------------------------
# device program
# ---------------------------------------------------------------------------

def _mid_bcast(ap, n, axis=1):
    """insert a step-0 dim of size n at position `axis` (free dims only)."""
    ap = ap.unsqueeze(axis)
    newap = list(ap.ap)
    newap[axis] = [0, n]
    return dataclasses.replace(ap, ap=newap)


def _build(cfg):
    nchb = list(cfg["nchb"])
    totch = sum(nchb)
    choff = np.concatenate([[0], np.cumsum(nchb)]).astype(int)
    NCHMAX = max(nchb)

    nc = bacc.Bacc("TRN2", target_bir_lowering=False, debug=False,
                   num_devices=NCORES)

    def din(name, shape, dt=FP32):
        return nc.dram_tensor(name, shape, dt, kind="ExternalInput")

    xtbl = din("xtbl", [N, FIN], BF16)
    idx1 = din("idx1", [128, totch * 8], I16)
    idx2 = din("idx2", [128, totch * 8], I16)
    gdst = din("gdst", [128, totch])
    wtab = din("wtab", [128, totch], BF16)
    cnt = din("cnt", [1, NCH], I32)
    padmask = din("padmask", [128, NCH])
    xT = din("xT", [FIN, NPAD], BF16)
    w1relT = din("w1relT", [FIN, HPAD], BF16)
    w1rootT = din("w1rootT", [FIN, HPAD], BF16)
    w2relT = din("w2relT", [HPAD, HPAD], BF16)
    w2rootT = din("w2rootT", [HPAD, HPAD], BF16)
    b1r = din("b1r", [128, HPAD])
    b2r = din("b2r", [128, HPAD])
    p1r = din("p1r", [128, HPAD])
    p2r = din("p2r", [128, HPAD])
    iota128 = din("iota128", [128, 128])
    iotaB = din("iotaB", [128, NBINS])
    ident = din("ident", [128, 128])
    identbf = din("identbf", [128, 128], BF16)
    ones1x128 = din("ones1x128", [1, 128])
    ones128 = din("ones128", [128, 128])
    onesP = din("onesP", [128, 1])
    onesPbf = din("onesPbf", [128, 1], BF16)
    l1T = din("l1T", [1024, 256], BF16)
    b1h = din("b1h", [128, 2])
    l2T = din("l2T", [256, 4096], BF16)
    b2cols = din("b2cols", [128, 32])
    l3T = din("l3T", [4096, 128], BF16)
    b3row = din("b3row", [1, 128])

    out = nc.dram_tensor("out", [1, NOUT], FP32, kind="ExternalOutput")

    RG = [list(range(NCORES))]

    with tile.TileContext(nc) as tc:
        with (
            tc.tile_pool(name="const", bufs=1) as cp,
            tc.tile_pool(name="gather", bufs=2) as gp,
            tc.tile_pool(name="work", bufs=1) as wp,
            tc.tile_pool(name="big", bufs=1) as bigp,
            tc.tile_pool(name="psA", bufs=2, space="PSUM") as psA,
            tc.tile_pool(name="psB", bufs=2, space="PSUM") as psB,
            tc.tile_pool(name="psS", bufs=1, space="PSUM") as psS,
            tc.tile_pool(name="dram", bufs=1, space="DRAM") as dr,
        ):
            def load(src, dt=FP32, tag=None):
                tl = cp.tile(list(src.shape), dt, tag=tag or src.name)
                nc.sync.dma_start(tl[:], src[:])
                return tl

            idx1_t = load(idx1, I16)
            idx2_t = load(idx2, I16)
            gdst_t = load(gdst)
            wtab_t = load(wtab, BF16)
            cnt_t = load(cnt, I32)
            pad_t = load(padmask)
            io_t = load(iota128)
            iob_t = load(iotaB)
            id_t = load(ident)
            idbf_t = load(identbf, BF16)
            ones_t = load(ones1x128)
            ones128_t = load(ones128)
            onesP_t = load(onesP)
            onesPbf_t = load(onesPbf, BF16)
            b1_t = load(b1r)
            b2_t = load(b2r)
            p1_t = load(p1r)
            p2_t = load(p2r)

            def load_chunks(src, nchunks, cols, tag, dt=BF16):
                ts = []
                for k in range(nchunks):
                    t = cp.tile([128, cols], dt, tag=f"{tag}{k}")
                    nc.sync.dma_start(t[:], src[k * 128:(k + 1) * 128, :cols])
                    ts.append(t)
                return ts

            w1rel_t = load_chunks(w1relT, 2, HPAD, "w1rel")
            w1root_t = load_chunks(w1rootT, 2, HPAD, "w1root")
            w2rel_t = load_chunks(w2relT, 4, HPAD, "w2rel")
            w2root_t = load_chunks(w2rootT, 4, HPAD, "w2root")
            xT_t = load_chunks(xT, 2, NPAD, "xTc")

            # DRAM internal tiles
            zsh1 = dr.tile([NPAD, 1], FP32)
            zag1 = dr.tile([NCORES * NPAD, 1], FP32, addr_space="Shared")
            zsh2 = dr.tile([NPAD, 1], FP32)
            zag2 = dr.tile([NCORES * NPAD, 1], FP32, addr_space="Shared")
            g1sh = dr.tile([NPAD, HPAD], BF16)
            g1ag = dr.tile([NCORES * NPAD, HPAD], BF16, addr_space="Shared")
            ro1in = dr.tile([2, HPAD], FP32)
            ro1ag = dr.tile([2 * NCORES, HPAD], FP32, addr_space="Shared")
            ro2in = dr.tile([2, HPAD], FP32)
            ro2ag = dr.tile([2 * NCORES, HPAD], FP32, addr_space="Shared")
            z2in = dr.tile([1, 4096], FP32)
            z2ar = dr.tile([1, 4096], FP32, addr_space="Shared")

            # per-bin edge counts: full (pads gather row 0 with w=0)
            cnt_regs = [nchb[b] * 128 for b in range(NCH)]

            # gather tiles (memset both ring buffers once: pad slots must
            # never hold NaN garbage)
            for _ in range(2):
                t1 = gp.tile([128, NCHMAX, FIN], BF16, tag="gathL1")
                t2 = gp.tile([128, NCHMAX, HPAD], BF16, tag="gathL2")
                nc.vector.memset(t1[:], 0.0)
                nc.vector.memset(t2[:], 0.0)

            # ---------------- conv layer -----------------------------------
            def conv_layer(F, pieces, idx_t, wrel_t, wroot_t, rootT_t, b_t,
                           p_t, h_all, z_all, gtag, lname):
                """per bin: gather -> weighted one-hot -> PE scatter-add ->
                transpose -> dense -> relu + score. pieces(b) yields
                (table_ap, chunk_lo, chunk_hi) gather pieces for bin b."""
                nfc = F // 128
                for b in range(NCH):
                    nch = nchb[b]
                    co = int(choff[b])
                    gt = gp.tile([128, NCHMAX, F], BF16, tag=gtag)
                    # split into <=1024-index calls (ucode-tested size)
                    for tbl_ap, c0, c1 in pieces(b):
                        for j0 in range(c0, c1, 8):
                            j1 = min(c1, j0 + 8)
                            nc.gpsimd.dma_gather(
                                gt[:, j0:j1, :], tbl_ap,
                                idx_t[:, (co + j0) * 8:(co + j1) * 8],
                                (j1 - j0) * 128, (j1 - j0) * 128, F)
                    # weighted one-hot [128, nch, 128]
                    oh = wp.tile([128, NCHMAX, 128], BF16, tag="ohw", bufs=2)
                    nc.vector.tensor_tensor(
                        out=oh[:, :nch, :],
                        in0=gdst_t[:, co:co + nch].unsqueeze(2)
                            .broadcast_to([128, nch, 128]),
                        in1=_mid_bcast(io_t[:], nch), op=OP.is_equal)
                    nc.vector.tensor_tensor(
                        out=oh[:, :nch, :], in0=oh[:, :nch, :],
                        in1=wtab_t[:, co:co + nch].unsqueeze(2)
                            .broadcast_to([128, nch, 128]), op=OP.mult)
                    # agg[dst, F] += oh.T @ gathered
                    agg_ps = psA.tile([128, HPAD], FP32, tag="aggps")
                    for c in range(nch):
                        nc.tensor.matmul(
                            out=agg_ps[:, :F], lhsT=oh[:, c, :],
                            rhs=gt[:, c, :],
                            start=(c == 0), stop=(c == nch - 1))
                    agg_sb = wp.tile([128, HPAD], BF16, tag="aggsb", bufs=2)
                    nc.vector.tensor_copy(agg_sb[:, :F], agg_ps[:, :F])
                    # transpose to [F, dst]
                    aggT = wp.tile([128, 4, 128], BF16, tag="aggT", bufs=2)
                    for fc in range(nfc):
                        tp = psB.tile([128, 128], BF16, tag="trp")
                        nc.tensor.transpose(
                            out=tp[:], in_=agg_sb[:, fc * 128:(fc + 1) * 128],
                            identity=idbf_t[:])
                        nc.vector.tensor_copy(aggT[:, fc, :], tp[:])
                    # dense: h = relu(aggT.T @ wrelT + root.T @ wrootT + b)
                    hp = psB.tile([128, HPAD], FP32, tag="hps")
                    for fc in range(nfc):
                        nc.tensor.matmul(
                            out=hp[:], lhsT=aggT[:, fc, :], rhs=wrel_t[fc][:],
                            start=(fc == 0), stop=False)
                    nroot = len(rootT_t)
                    for fc in range(nroot):
                        nc.tensor.matmul(
                            out=hp[:], lhsT=rootT_t[fc][:, b * 128:(b + 1) * 128],
                            rhs=wroot_t[fc][:], start=False,
                            stop=(fc == nroot - 1))
                    hc = h_all[:, b * HPAD:(b + 1) * HPAD]
                    nc.vector.tensor_tensor(out=hc, in0=hp[:], in1=b_t[:],
                                            op=OP.add)
                    nc.scalar.activation(hc, hc, ACT.Relu)
                    scr = wp.tile([128, HPAD], FP32, tag="scr", bufs=2)
                    nc.vector.tensor_tensor(out=scr[:], in0=hc, in1=p_t[:],
                                            op=OP.mult)
                    nc.vector.tensor_reduce(out=z_all[:, b:b + 1], in_=scr[:],
                                            op=OP.add, axis=AX.X)

            # ---------------- histogram k-th threshold ---------------------
            def topk_tau(zag, k, lname):
                """returns [128,1] tile with the k-th-largest threshold."""
                nfree = NCORES * NPAD // 128
                zt = wp.tile([128, nfree], FP32, tag="zt")
                nc.sync.dma_start(
                    zt[:], zag[:].rearrange("(p f) o -> p (f o)", p=128))
                ztb = wp.tile([128, nfree], BF16, tag="ztb")
                nc.vector.tensor_copy(ztb[:], zt[:])
                # min over real entries (pads are -1e30), max overall
                mm = wp.tile([128, 2], FP32, tag="mm")
                msk = wp.tile([128, nfree], FP32, tag="hmsk")
                nc.vector.tensor_scalar(msk[:], zt[:], -1e29, 2e30, OP.is_lt,
                                        OP.mult)
                nc.vector.tensor_tensor(out=msk[:], in0=msk[:], in1=zt[:],
                                        op=OP.add)
                nc.vector.tensor_reduce(out=mm[:, 0:1], in_=msk[:], op=OP.min,
                                        axis=AX.X)
                nc.vector.tensor_reduce(out=mm[:, 1:2], in_=zt[:], op=OP.max,
                                        axis=AX.X)
                lw = wp.tile([1, 2], FP32, tag="lw")  # [lo, w]
                mmT = wp.tile([1, 2, 128], FP32, tag="mmTs")
                for col in range(2):
                    mmT_ps = psS.tile([1, 128], FP32, tag="small")
                    nc.tensor.transpose(out=mmT_ps[:], in_=mm[:, col:col + 1],
                                        identity=id_t[:])
                    nc.vector.tensor_copy(mmT[:, col, :], mmT_ps[:])
                nc.vector.tensor_reduce(out=lw[:, 0:1], in_=mmT[:, 0, :],
                                        op=OP.min, axis=AX.X)
                nc.vector.tensor_reduce(out=lw[:, 1:2], in_=mmT[:, 1, :],
                                        op=OP.max, axis=AX.X)
                nc.vector.tensor_scalar_add(lw[:, 0:1], lw[:, 0:1], -1e-3)
                nc.vector.tensor_scalar_add(lw[:, 1:2], lw[:, 1:2], 1e-3)
                nc.vector.tensor_tensor(out=lw[:, 1:2], in0=lw[:, 1:2],
                                        in1=lw[:, 0:1], op=OP.subtract)
                nc.vector.tensor_scalar_mul(lw[:, 1:2], lw[:, 1:2], 1.0 / NBINS)

                # broadcast [lo, w] to all partitions once; all later stage
                # math is replicated on [128, *] tiles
                lwr_ps = psS.tile([128, 2], FP32, tag="small")
                nc.tensor.matmul(out=lwr_ps[:], lhsT=ones_t[:], rhs=lw[:],
                                 start=True, stop=True)
                lwr = wp.tile([128, 2], FP32, tag=f"lwr{lname}")
                nc.vector.tensor_copy(lwr[:], lwr_ps[:])

                for st in range(NSTAGES):
                    tt = wp.tile([128, NBINS], BF16, tag="tt")
                    nc.vector.tensor_scalar(tt[:], iob_t[:], lwr[:, 1:2],
                                            lwr[:, 0:1], OP.mult, OP.add)
                    # S[p, j, n] = (z[p, n] >= t[p, j]); contiguous inner
                    S = wp.tile([128, NBINS, nfree], BF16, tag="S")
                    nc.vector.tensor_tensor(
                        out=S[:], in0=_mid_bcast(ztb[:], NBINS),
                        in1=tt[:].unsqueeze(2)
                            .broadcast_to([128, NBINS, nfree]),
                        op=OP.is_ge)
                    cntp = wp.tile([128, NBINS], FP32, tag="cntp")
                    nc.vector.tensor_reduce(out=cntp[:], in_=S[:],
                                            op=OP.add, axis=AX.X)
                    # replicate totals to every partition in one matmul
                    cntr_ps = psS.tile([128, NBINS], FP32, tag="small")
                    nc.tensor.matmul(out=cntr_ps[:], lhsT=ones128_t[:],
                                     rhs=cntp[:], start=True, stop=True)
                    # fl = (count >= k), with margin for fp32r count noise
                    fl = wp.tile([128, NBINS], FP32, tag="fl")
                    js = wp.tile([128, 1], FP32, tag="js")
                    nc.vector.tensor_scalar(fl[:], cntr_ps[:], float(k) - 0.5,
                                            None, OP.is_ge)
                    nc.vector.tensor_reduce(out=js[:], in_=fl[:], op=OP.add,
                                            axis=AX.X)
                    nc.vector.tensor_scalar_add(js[:], js[:], -1.0)
                    nc.vector.tensor_scalar(lwr[:, 0:1], js[:], lwr[:, 1:2],
                                            lwr[:, 0:1], OP.mult, OP.add)
                    if st != NSTAGES - 1:
                        nc.vector.tensor_scalar_mul(lwr[:, 1:2], lwr[:, 1:2],
                                                    1.0 / NBINS)
                return lwr

            def inv_norm_b(p_t, lname):
                """[128,1] broadcast of 1/||p||."""
                sq = wp.tile([1, HPAD], FP32, tag="pnsq")
                nc.vector.tensor_tensor(out=sq[:], in0=p_t[0:1, :],
                                        in1=p_t[0:1, :], op=OP.mult)
                n2 = wp.tile([1, 1], FP32, tag="pn2")
                nc.vector.tensor_reduce(out=n2[:], in_=sq[:], op=OP.add,
                                        axis=AX.X)
                nc.scalar.activation(n2[:], n2[:], ACT.Sqrt)
                nc.vector.reciprocal(n2[:], n2[:])
                ib_ps = psS.tile([128, 1], FP32, tag="small")
                nc.tensor.matmul(out=ib_ps[:], lhsT=ones_t[:], rhs=n2[:],
                                 start=True, stop=True)
                ib = wp.tile([128, 1], FP32, tag=f"invbs{lname}")
                nc.vector.tensor_copy(ib[:], ib_ps[:])
                return ib

            # ======================= layer 1 ===============================
            h1 = bigp.tile([128, NCH * HPAD], FP32, tag="h_all")
            z1 = wp.tile([128, NCH], FP32, tag="z1")
            conv_layer(FIN, lambda b: [(xtbl[:], 0, nchb[b])],
                       idx1_t, w1rel_t, w1root_t, xT_t,
                       b1_t, p1_t, h1[:], z1[:], "gathL1", "l1")

            inv1b = inv_norm_b(p1_t, "l1")
            s1 = wp.tile([128, NCH], FP32, tag="s1")
            nc.scalar.activation(s1[:], z1[:], ACT.Tanh, scale=inv1b[:, 0:1])

            pm30 = wp.tile([128, NCH], FP32, tag="pm30")
            nc.vector.tensor_scalar(pm30[:], pad_t[:], 1.0, BIG, OP.subtract,
                                    OP.mult)
            zm1 = wp.tile([128, NCH], FP32, tag="zm1")
            nc.vector.tensor_tensor(out=zm1[:], in0=z1[:], in1=pad_t[:],
                                    op=OP.mult)
            nc.vector.tensor_tensor(out=zm1[:], in0=zm1[:], in1=pm30[:],
                                    op=OP.add)
            nc.sync.dma_start(
                zsh1[:].rearrange("(b p) o -> p (b o)", p=128), zm1[:])
            nc.gpsimd.collective_compute(
                "AllGather", OP.bypass, replica_groups=RG,
                ins=[zsh1[:]], outs=[zag1[:]])

            tau1b = topk_tau(zag1, K1, "l1")
            kp1 = wp.tile([128, NCH], FP32, tag="kp1")
            nc.vector.tensor_scalar(kp1[:], zm1[:], tau1b[:, 0:1], None,
                                    OP.is_ge)
            a1 = wp.tile([128, NCH], FP32, tag="a1")
            nc.vector.tensor_tensor(out=a1[:], in0=s1[:], in1=kp1[:],
                                    op=OP.mult)
            km30 = wp.tile([128, NCH], FP32, tag="km30")
            nc.vector.tensor_scalar(km30[:], kp1[:], 1.0, BIG, OP.subtract,
                                    OP.mult)

            # g1 (+ masked transpose) + readout 1
            gmT1 = [bigp.tile([128, NPAD], BF16, tag=f"gmT{fc}",
                              name=f"gmT1_{fc}")
                    for fc in range(4)]
            ro1s_ps = psS.tile([1, HPAD], FP32, tag="rosum")
            # pass 1: only what gates the g1 AllGather (table rows + sums)
            for b in range(NCH):
                hc = h1[:, b * HPAD:(b + 1) * HPAD]
                g1c = wp.tile([128, HPAD], BF16, tag="g1c", bufs=2)
                nc.vector.tensor_scalar(g1c[:], hc, a1[:, b:b + 1], None,
                                        OP.mult)
                nc.sync.dma_start(g1sh[b * 128:(b + 1) * 128, :], g1c[:])
                nc.tensor.matmul(out=ro1s_ps[:], lhsT=onesPbf_t[:], rhs=g1c[:],
                                 start=(b == 0), stop=(b == NCH - 1))
            nc.gpsimd.collective_compute(
                "AllGather", OP.bypass, replica_groups=RG,
                ins=[g1sh[:]], outs=[g1ag[:]])
            # pass 2: masked transposes (L2 root + readout max), off the
            # AllGather critical path
            for b in range(NCH):
                hc = h1[:, b * HPAD:(b + 1) * HPAD]
                gmc = wp.tile([128, HPAD], BF16, tag="gmc", bufs=2)
                nc.vector.tensor_scalar(gmc[:], hc, a1[:, b:b + 1],
                                        km30[:, b:b + 1], OP.mult, OP.add)
                for fc in range(4):
                    tp = psB.tile([128, 128], BF16, tag="trp")
                    nc.tensor.transpose(out=tp[:],
                                        in_=gmc[:, fc * 128:(fc + 1) * 128],
                                        identity=idbf_t[:])
                    nc.vector.tensor_copy(gmT1[fc][:, b * 128:(b + 1) * 128],
                                          tp[:])

            m1T = wp.tile([128, 4], FP32, tag="m1T")
            for fc in range(4):
                nc.vector.tensor_reduce(out=m1T[:, fc:fc + 1], in_=gmT1[fc][:],
                                        op=OP.max, axis=AX.X)
            ro1s = wp.tile([1, HPAD], FP32, tag="ro1s")
            nc.vector.tensor_copy(ro1s[:], ro1s_ps[:])
            nc.sync.dma_start(ro1in[0:1, :], ro1s[:])
            nc.sync.dma_start(
                ro1in[1:2, :].rearrange("o (c p) -> p (o c)", p=128), m1T[:])
            nc.gpsimd.collective_compute(
                "AllGather", OP.bypass, replica_groups=RG,
                ins=[ro1in[:]], outs=[ro1ag[:]])

            # ======================= layer 2 ===============================
            h2 = bigp.tile([128, NCH * HPAD], FP32, tag="h_all")
            z2 = wp.tile([128, NCH], FP32, tag="z2")
            conv_layer(HPAD, lambda b: [(g1ag[:], 0, nchb[b])],
                       idx2_t, w2rel_t, w2root_t, gmT1,
                       b2_t, p2_t, h2[:], z2[:], "gathL2", "l2")

            inv2b = inv_norm_b(p2_t, "l2")
            s2 = wp.tile([128, NCH], FP32, tag="s2")
            nc.scalar.activation(s2[:], z2[:], ACT.Tanh, scale=inv2b[:, 0:1])
            zm2 = wp.tile([128, NCH], FP32, tag="zm2")
            nc.vector.tensor_tensor(out=zm2[:], in0=z2[:], in1=kp1[:],
                                    op=OP.mult)
            nc.vector.tensor_tensor(out=zm2[:], in0=zm2[:], in1=km30[:],
                                    op=OP.add)
            nc.sync.dma_start(
                zsh2[:].rearrange("(b p) o -> p (b o)", p=128), zm2[:])
            nc.gpsimd.collective_compute(
                "AllGather", OP.bypass, replica_groups=RG,
                ins=[zsh2[:]], outs=[zag2[:]])

            tau2b = topk_tau(zag2, K2, "l2")
            kp2 = wp.tile([128, NCH], FP32, tag="kp2")
            nc.vector.tensor_scalar(kp2[:], zm2[:], tau2b[:, 0:1], None,
                                    OP.is_ge)
            a2 = wp.tile([128, NCH], FP32, tag="a2")
            nc.vector.tensor_tensor(out=a2[:], in0=s2[:], in1=kp2[:],
                                    op=OP.mult)
            km30b = wp.tile([128, NCH], FP32, tag="km30b")
            nc.vector.tensor_scalar(km30b[:], kp2[:], 1.0, BIG, OP.subtract,
                                    OP.mult)

            ro2s_ps = psS.tile([1, HPAD], FP32, tag="rosum")
            m2T = wp.tile([128, 4], FP32, tag="m2T")
            nc.vector.memset(m2T[:], -1e30)
            for b in range(NCH):
                hc = h2[:, b * HPAD:(b + 1) * HPAD]
                g2c = wp.tile([128, HPAD], BF16, tag="g1c", bufs=2)
                nc.vector.tensor_scalar(g2c[:], hc, a2[:, b:b + 1], None,
                                        OP.mult)
                nc.tensor.matmul(out=ro2s_ps[:], lhsT=onesPbf_t[:], rhs=g2c[:],
                                 start=(b == 0), stop=(b == NCH - 1))
                gmc = wp.tile([128, HPAD], BF16, tag="gmc", bufs=2)
                nc.vector.tensor_scalar(gmc[:], hc, a2[:, b:b + 1],
                                        km30b[:, b:b + 1], OP.mult, OP.add)
                for fc in range(4):
                    tp = psB.tile([128, 128], BF16, tag="trp")
                    nc.tensor.transpose(out=tp[:],
                                        in_=gmc[:, fc * 128:(fc + 1) * 128],
                                        identity=idbf_t[:])
                    red = wp.tile([128, 1], FP32, tag="redm", bufs=2)
                    nc.vector.tensor_reduce(out=red[:], in_=tp[:], op=OP.max,
                                            axis=AX.X)
                    nc.vector.tensor_tensor(out=m2T[:, fc:fc + 1],
                                            in0=m2T[:, fc:fc + 1], in1=red[:],
                                            op=OP.max)
            ro2s = wp.tile([1, HPAD], FP32, tag="ro2s")
            nc.vector.tensor_copy(ro2s[:], ro2s_ps[:])
            nc.sync.dma_start(ro2in[0:1, :], ro2s[:])
            nc.sync.dma_start(
                ro2in[1:2, :].rearrange("o (c p) -> p (o c)", p=128), m2T[:])
            nc.gpsimd.collective_compute(
                "AllGather", OP.bypass, replica_groups=RG,
                ins=[ro2in[:]], outs=[ro2ag[:]])

            # ======================= readout combine + head ================
            def combine(roag, kdiv, mxout, mnout):
                """[16, HPAD] AG -> maxT [128,4], meanT [128,4] (transposed)."""
                sums = wp.tile([128, 2 * NCORES, 4], FP32, tag="cmb")
                nc.sync.dma_start(
                    sums[:],
                    roag[:].rearrange("r (c p) -> p (r c)", p=128))
                s_ap = sums[:].rearrange("p (s t) c -> p c t s", t=2)
                nc.vector.tensor_reduce(out=mnout[:], in_=s_ap[:, :, 0, :],
                                        op=OP.add, axis=AX.X)
                nc.vector.tensor_reduce(out=mxout[:], in_=s_ap[:, :, 1, :],
                                        op=OP.max, axis=AX.X)
                nc.vector.tensor_scalar_mul(mnout[:], mnout[:], 1.0 / kdiv)

            mx1 = wp.tile([128, 4], FP32, tag="mx1")
            mn1 = wp.tile([128, 4], FP32, tag="mn1")
            combine(ro1ag, K1, mx1, mn1)
            mx2 = wp.tile([128, 4], FP32, tag="mx2")
            mn2 = wp.tile([128, 4], FP32, tag="mn2")
            combine(ro2ag, K2, mx2, mn2)

            zT = wp.tile([128, 8], BF16, tag="zT")
            nc.vector.tensor_tensor(out=zT[:, 0:4], in0=mx1[:], in1=mx2[:],
                                    op=OP.add)
            nc.vector.tensor_tensor(out=zT[:, 4:8], in0=mn1[:], in1=mn2[:],
                                    op=OP.add)

            # lin1: z1cols [128, 2] = relu(l1T.T @ zT + b1h), row-shard
            l1_t = load_chunks(l1T, 8, 256, "l1T")
            b1h_t = load(b1h)
            z1cols = wp.tile([128, 2], BF16, tag="z1cols")
            for m in range(2):
                o1p = psS.tile([128, 1], FP32, tag="small")
                for t in range(8):
                    nc.tensor.matmul(out=o1p[:],
                                     lhsT=l1_t[t][:, m * 128:(m + 1) * 128],
                                     rhs=zT[:, t:t + 1],
                                     start=(t == 0), stop=(t == 7))
                nc.scalar.activation(z1cols[:, m:m + 1], o1p[:], ACT.Relu,
                                     bias=b1h_t[:, m:m + 1])

            # lin2 partial contraction: z2p [1, 4096] = l2T.T @ z1cols
            l2_t = load_chunks(l2T, 2, 4096, "l2Tc")
            z2p = wp.tile([1, 4096], FP32, tag="z2p")
            for s in range(8):
                o2p = psS.tile([1, 512], FP32, tag="rosum")
                for t in range(2):
                    nc.tensor.matmul(
                        out=o2p[:], lhsT=z1cols[:, t:t + 1],
                        rhs=l2_t[t][:, s * 512:(s + 1) * 512],
                        start=(t == 0), stop=(t == 1))
                nc.vector.tensor_copy(z2p[:, s * 512:(s + 1) * 512], o2p[:])
            nc.sync.dma_start(z2in[:], z2p[:])
            nc.gpsimd.collective_compute(
                "AllReduce", OP.add, replica_groups=RG,
                ins=[z2in[:]], outs=[z2ar[:]])

            # z2cols [128, 32] = relu(z2ar + b2), column-chunk layout
            b2c_t = load(b2cols)
            z2cols = wp.tile([128, 32], BF16, tag="z2cols")
            z2f = wp.tile([128, 32], FP32, tag="z2f")
            nc.sync.dma_start(
                z2f[:], z2ar[:].rearrange("o (c p) -> p (o c)", p=128))
            nc.vector.tensor_tensor(out=z2f[:], in0=z2f[:], in1=b2c_t[:],
                                    op=OP.add)
            nc.vector.tensor_scalar_max(z2cols[:], z2f[:], 0.0)

            # lin3 replicated: out [1, 100] = l3T.T @ z2cols + b3
            l3_t = load_chunks(l3T, 32, 128, "l3T")
            b3_t = load(b3row)
            o3p = psS.tile([1, 128], FP32, tag="small")
            for t in range(32):
                nc.tensor.matmul(out=o3p[:], lhsT=z2cols[:, t:t + 1],
                                 rhs=l3_t[t][:], start=(t == 0),
                                 stop=(t == 31))
            fin = wp.tile([1, 128], FP32, tag="fin")
            nc.vector.tensor_tensor(out=fin[:], in0=o3p[:], in1=b3_t[:],
                                    op=OP.add)
            nc.scalar.activation(fin[:], fin[:], ACT.Sigmoid)
            nc.sync.dma_start(out[:], fin[:, :NOUT])

    nc.compile()
    return nc


# ---------------------------------------------------------------------------
# entry point
# ---------------------------------------------------------------------------

_CACHE = {}
TRACE = False


def kernel(**inputs):
    cfg, bins = _pack(inputs["edge_src"], inputs["edge_dst"],
                      inputs["edge_weight"])
    key = cfg["nchb"]
    if key not in _CACHE:
        _CACHE[key] = _build(cfg)
    nc = _CACHE[key]
    in_maps = _host_inputs(inputs, cfg, bins)
    res = bass_utils.run_bass_kernel_spmd(
        nc, in_maps, core_ids=list(range(NCORES)), trace=TRACE)
    kernel.last_results = res
    return res.results[0]["out"]


if __name__ == "__main__":
    dat = np.load("/tmp/inputs.npz")
    inputs = {k: dat[k] for k in dat.files}
    got = kernel(**inputs)
    exp = np.load("/tmp/expected.npy")
    err = np.abs(got - exp).max()
    rel = err / np.abs(exp).max()
    print("out[0,:6] =", got[0, :6])
    print("exp[0,:6] =", exp[0, :6])
    print("max abs err:", err, "rel:", rel)
